# revision 11
# baseline (speedup 1.0000x reference)
"""Trainium2 Bass kernel for AtomExposureGNN (3-layer GCN, N=50000, E=800000).

Distribution: nodes are partitioned across 8 NeuronCores (graph parallel).
Host-side (free, untimed): self-loops, degrees, symmetric-norm factorization,
degree-balanced node permutation, per-dst-block padded neighbor index lists.
Device: per layer, each core gathers message rows hw[src] (dma_gather from its
DRAM copy of the table), accumulates them per 128-node dst block on the
TensorEngine (PSUM, with dis[dst] folded into the stationary diag operand),
applies the folded batchnorm affine + relu + residual, computes the next
table shard hw = (dis*h) @ Wc', and AllGathers shards into the next table.

Key algebra: with dis = deg^-1/2,
  agg[d] = sum_e dis[s]dis[d] (h Wc)[s] = dis[d] * sum_e ((dis*h) Wc)[s]
  bn(agg+bc) = agg*a + b  with a,b per-channel  ->  a folds into Wc columns.
"""

import numpy as np
import ml_dtypes

# ---- problem constants (hardcoded per harness contract) ----
N, E, DIN, H, L = 50000, 800000, 64, 128, 3
NCORES = 8
NPC = 6272            # nodes per core (49 * 128), >= ceil(50000/8)=6250
NB = NPC // 128       # 49 dst blocks per core
NTOT = NPC * NCORES   # 50176 global slots
HALF = NTOT // 2      # region A = slots [0, HALF) (cores 0-3), B = rest
ZTOK = 6250           # zero-token: first pad slot of core 0 / core 4 (local id)
EPS = 1e-5
TILE_CAP = 112        # max message tiles (128 tokens each) per gather group
BF16 = ml_dtypes.bfloat16

LAST_EXEC_NS = None
TRACE = False
TRACE_DIR = None
DEBUG_NO_COLLECTIVE = False   # replace AllGather with local DMA (wrong numerics)
GATHER_MAX_IDX = 32768        # split dma_gather calls to at most this many idxs

_CACHE = {}


# ============================ host preprocessing ============================

def _prep(x, edge_index, W_in, b_in, Wc, bc, gamma, beta, rmean, rvar,
          W1, b1, W2, b2):
    x = np.asarray(x, np.float32)
    ei = np.asarray(edge_index, np.int64)
    src = np.concatenate([ei[0], np.arange(N, dtype=np.int64)])
    dst = np.concatenate([ei[1], np.arange(N, dtype=np.int64)])

    deg = np.bincount(dst, minlength=N)
    dis = (1.0 / np.sqrt(deg.astype(np.float64))).astype(np.float32)

    # node -> core: stripe the degree-sorted order for balance
    order = np.argsort(deg, kind="stable")
    core_of = np.empty(N, np.int64)
    core_of[order] = np.arange(N) % NCORES
    region_a = core_of < (NCORES // 2)

    # per-node count of in-edges whose src lands in region A
    a_cnt = np.bincount(dst[region_a[src]], minlength=N)

    # per-core local slot order: (deg, a_cnt) lexsort keeps per-block padding tight
    slot_of = np.empty(N, np.int64)
    for c in range(NCORES):
        nodes_c = np.flatnonzero(core_of == c)
        o = np.lexsort((a_cnt[nodes_c], deg[nodes_c]))
        slot_of[nodes_c[o]] = c * NPC + np.arange(nodes_c.size)
    node_of = np.full(NTOT, -1, np.int64)
    node_of[slot_of] = np.arange(N)

    gd = slot_of[dst]
    gs = slot_of[src]
    reg = (gs >= HALF).astype(np.int64)
    tok = gs - reg * HALF
    core_e = gd // NPC
    blk = (gd % NPC) // 128
    par = gd % 128

    key = ((core_e * NB + blk) * 2 + reg) * 128 + par
    cnt = np.bincount(key, minlength=NCORES * NB * 2 * 128)
    cnt = cnt.reshape(NCORES, NB, 2, 128)
    D_A = cnt[:, :, 0, :].max(axis=(0, 2)).astype(np.int64)   # [NB]
    D_B = cnt[:, :, 1, :].max(axis=(0, 2)).astype(np.int64)

    # contiguous greedy grouping of blocks for batched gathers
    dsum = D_A + D_B
    groups, cur, acc = [], [], 0
    for b in range(NB):
        if cur and acc + dsum[b] > TILE_CAP:
            groups.append(cur)
            cur, acc = [], 0
        cur.append(b)
        acc += int(dsum[b])
    groups.append(cur)

    # idx sequence layout: per group, A spans of its blocks then B spans
    offA = np.zeros(NB, np.int64)
    offB = np.zeros(NB, np.int64)
    off = 0
    for gblocks in groups:
        for b in gblocks:
            offA[b] = off
            off += D_A[b] * 128
        for b in gblocks:
            offB[b] = off
            off += D_B[b] * 128
    ltot = int(off)
    assert ltot % 128 == 0

    # fill token slots: edge with within-(core,blk,reg,par) rank j goes to
    # sequence position off{A,B}[blk] + j*128 + par
    ordk = np.argsort(key, kind="stable")
    ks = key[ordk]
    newrun = np.r_[True, ks[1:] != ks[:-1]]
    runstart = np.flatnonzero(newrun)
    runid = np.cumsum(newrun) - 1
    cc = np.arange(ks.size) - runstart[runid]

    idx_all = np.full((NCORES, ltot), ZTOK, np.int16)
    pos = np.where(reg[ordk] == 0, offA[blk[ordk]], offB[blk[ordk]])
    pos = pos + cc * 128 + par[ordk]
    idx_all[core_e[ordk], pos] = tok[ordk].astype(np.int16)

    idx_sb = np.empty((NCORES, 128, ltot // 16), np.int16)
    for c in range(NCORES):
        w = idx_all[c].reshape(ltot // 16, 16).T
        idx_sb[c] = np.tile(w, (8, 1))

    # per-slot dis / x
    dis_slot = np.zeros(NTOT, np.float32)
    dis_slot[slot_of] = dis
    xp = np.zeros((NTOT, DIN), np.float32)
    xp[slot_of] = x

    dcol = np.empty((NCORES, 128, NB), np.float32)
    ddiag = np.zeros((NCORES, 128, NB, 128), np.float32)
    xT = np.empty((NCORES, DIN, NPC), np.float32)
    p_i = np.arange(128)
    for c in range(NCORES):
        d = dis_slot[c * NPC:(c + 1) * NPC].reshape(NB, 128)
        dcol[c] = d.T
        ddiag[c][p_i[:, None], np.arange(NB)[None, :], p_i[:, None]] = d.T
        xT[c] = xp[c * NPC:(c + 1) * NPC].T
    ddiag = ddiag.reshape(NCORES, 128, NB * 128).astype(BF16)

    # folded batchnorm affine
    rs = 1.0 / np.sqrt(np.asarray(rvar, np.float64) + EPS)
    a_l = (rs * np.asarray(gamma, np.float64)).astype(np.float32)       # [L,H]
    b_l = ((np.asarray(bc, np.float64) - np.asarray(rmean, np.float64))
           * rs * np.asarray(gamma, np.float64)
           + np.asarray(beta, np.float64)).astype(np.float32)           # [L,H]
    wcp = (np.asarray(Wc, np.float32) * a_l[:, None, :])                # [L,H,H]
    wc_sb = np.concatenate([wcp[i] for i in range(L)], axis=1).astype(BF16)
    brep = np.tile(b_l[None, :, None, :], (128, 1, 4, 1))
    brep = brep.reshape(128, L * 4 * H).astype(np.float32)

    assert (D_A + D_B >= 1).all()
    meta = dict(D_A=D_A, D_B=D_B, groups=groups, offA=offA, offB=offB,
                ltot=ltot, node_of=node_of, slot_of=slot_of)
    per_core = []
    for c in range(NCORES):
        per_core.append(dict(
            xT=np.ascontiguousarray(xT[c]),
            idx=np.ascontiguousarray(idx_sb[c]),
            ddiag=np.ascontiguousarray(ddiag[c]),
            dcol=np.ascontiguousarray(dcol[c]),
        ))
    shared = dict(
        win=np.asarray(W_in, np.float32).astype(BF16),                  # [64,128]
        wc=wc_sb,                                                       # [128,3H]
        w1=np.asarray(W1, np.float32).astype(BF16),                     # [128,64]
        w2=np.asarray(W2, np.float32).astype(BF16),                     # [64,1]
        bin=np.asarray(b_in, np.float32).reshape(H, 1),
        b1=np.asarray(b1, np.float32).reshape(H // 2, 1),
        brep=brep,
        ident=np.eye(128, dtype=np.float32).astype(BF16),
    )
    b2f = float(np.asarray(b2, np.float32).reshape(-1)[0])
    return meta, per_core, shared, b2f


# ============================ device kernel ============================

def _build(meta, b2f):
    import concourse.bass as bass
    import concourse.bacc as bacc
    import concourse.tile as tile
    import concourse.mybir as mybir
    from contextlib import ExitStack

    f32 = mybir.dt.float32
    bf16 = mybir.dt.bfloat16
    i16 = mybir.dt.int16
    RELU = mybir.ActivationFunctionType.Relu

    D_A, D_B = meta["D_A"], meta["D_B"]
    groups = meta["groups"]
    offA, offB = meta["offA"], meta["offB"]
    ltot = meta["ltot"]

    group_of = {}
    for g, gblocks in enumerate(groups):
        for b in gblocks:
            group_of[b] = g

    nc = bacc.Bacc("TRN2", target_bir_lowering=False, debug=False,
                   num_devices=NCORES)

    xT_d = nc.dram_tensor("xT", [DIN, NPC], f32, kind="ExternalInput")
    idx_d = nc.dram_tensor("idx", [128, ltot // 16], i16, kind="ExternalInput")
    ddiag_d = nc.dram_tensor("ddiag", [128, NB * 128], bf16, kind="ExternalInput")
    dcol_d = nc.dram_tensor("dcol", [128, NB], f32, kind="ExternalInput")
    win_d = nc.dram_tensor("win", [DIN, H], bf16, kind="ExternalInput")
    wc_d = nc.dram_tensor("wc", [H, L * H], bf16, kind="ExternalInput")
    w1_d = nc.dram_tensor("w1", [H, H // 2], bf16, kind="ExternalInput")
    w2_d = nc.dram_tensor("w2", [H // 2, 1], bf16, kind="ExternalInput")
    bin_d = nc.dram_tensor("bin", [H, 1], f32, kind="ExternalInput")
    b1_d = nc.dram_tensor("b1", [H // 2, 1], f32, kind="ExternalInput")
    brep_d = nc.dram_tensor("brep", [128, L * 4 * H], f32, kind="ExternalInput")
    ident_d = nc.dram_tensor("ident", [128, 128], bf16, kind="ExternalInput")
    out_d = nc.dram_tensor("out", [NB, 128], f32, kind="ExternalOutput")

    hwb = nc.dram_tensor("hwb", [NPC, H], bf16)
    tabs = [nc.dram_tensor(f"tab{i}", [NTOT, H], bf16, addr_space="Shared")
            for i in range(L)]
    rg = [list(range(NCORES))]

    with tile.TileContext(nc) as tc, ExitStack() as ctx:
        const = ctx.enter_context(tc.tile_pool(name="const", bufs=1))
        hpool = ctx.enter_context(tc.tile_pool(name="hst", bufs=1))
        msgp = ctx.enter_context(tc.tile_pool(name="msg", bufs=2))
        work = ctx.enter_context(tc.tile_pool(name="work", bufs=3))
        psagg = ctx.enter_context(
            tc.tile_pool(name="psagg", bufs=2, space="PSUM"))
        psw = ctx.enter_context(tc.tile_pool(name="psw", bufs=2, space="PSUM"))

        # ---- persistent SBUF residents ----
        idx_t = const.tile([128, ltot // 16], i16)
        nc.sync.dma_start(idx_t[:], idx_d[:])
        ddiag_t = const.tile([128, NB * 128], bf16)
        nc.sync.dma_start(ddiag_t[:], ddiag_d[:])
        dcol_t = const.tile([128, NB], f32)
        nc.sync.dma_start(dcol_t[:], dcol_d[:])
        brep_t = const.tile([128, L * 4 * H], f32)
        nc.sync.dma_start(brep_t[:], brep_d[:])
        win_t = const.tile([DIN, H], bf16)
        nc.sync.dma_start(win_t[:], win_d[:])
        wc_t = const.tile([H, L * H], bf16)
        nc.sync.dma_start(wc_t[:], wc_d[:])
        w1_t = const.tile([H, H // 2], bf16)
        nc.sync.dma_start(w1_t[:], w1_d[:])
        w2_t = const.tile([H // 2, 1], bf16)
        nc.sync.dma_start(w2_t[:], w2_d[:])
        bin_t = const.tile([H, 1], f32)
        nc.sync.dma_start(bin_t[:], bin_d[:])
        b1_t = const.tile([H // 2, 1], f32)
        nc.sync.dma_start(b1_t[:], b1_d[:])
        ident_t = const.tile([128, 128], bf16)
        nc.sync.dma_start(ident_t[:], ident_d[:])

        h_store = hpool.tile([128, NB, H], f32)

        # ---- layer 0: hw0 = dis * (relu(x W_in + b_in) @ Wc'0), own shard ----
        done = 0
        l0chunks = [512] * (NPC // 512)
        if NPC % 512:
            l0chunks.append(NPC % 512)
        for w in l0chunks:
            sl = slice(done, done + w)
            xt_b = work.tile([DIN, w], bf16, tag="xt")
            nc.gpsimd.dma_start(xt_b[:], xT_d[:, sl])       # f32 -> bf16 cast
            ps0 = psw.tile([H, w], f32, tag="pT")
            nc.tensor.matmul(ps0[:], win_t[:], xt_b[:], start=True, stop=True)
            h0 = work.tile([H, w], bf16, tag="h0")
            nc.scalar.activation(h0[:], ps0[:], RELU, bias=bin_t[:])
            for s in range(w // 128):
                b = done // 128 + s
                ps2 = psw.tile([128, H], f32, tag="p2")
                nc.tensor.matmul(ps2[:], h0[:, s * 128:(s + 1) * 128],
                                 wc_t[:, 0:H], start=True, stop=True)
                hw_t = work.tile([128, H], bf16, tag="hwt")
                nc.vector.tensor_scalar_mul(hw_t[:], ps2[:],
                                            dcol_t[:, b:b + 1])
                nc.sync.dma_start(hwb[b * 128:(b + 1) * 128, :], hw_t[:])
            done += w

        def allgather(dst):
            if DEBUG_NO_COLLECTIVE:
                nc.gpsimd.dma_start(dst[0:NPC, :], hwb[:])
            else:
                nc.gpsimd.collective_compute(
                    "AllGather", mybir.AluOpType.bypass, replica_groups=rg,
                    ins=[hwb[:]], outs=[dst[:]])

        allgather(tabs[0])

        # ---- GCN layers ----
        supers = [list(range(s, min(s + 4, NB))) for s in range(0, NB, 4)]

        for l in range(L):
            tabA = tabs[l][:HALF, :]
            tabB = tabs[l][HALF:, :]

            msgs = {}

            def issue_group(g):
                gblocks = groups[g]
                la = int(sum(D_A[b] for b in gblocks))
                lb = int(sum(D_B[b] for b in gblocks))
                mA = mB = None
                step = max(1, GATHER_MAX_IDX // 128)
                if la:
                    mA = msgp.tile([128, la, H], bf16, tag="mA")
                    c0 = int(offA[gblocks[0]]) // 16
                    for t0 in range(0, la, step):
                        tn = min(step, la - t0)
                        nc.gpsimd.dma_gather(
                            mA[:, t0:t0 + tn, :], tabA,
                            idx_t[:, c0 + t0 * 8:c0 + (t0 + tn) * 8],
                            num_idxs=tn * 128, num_idxs_reg=tn * 128,
                            elem_size=H, elem_step=H)
                if lb:
                    mB = msgp.tile([128, lb, H], bf16, tag="mB")
                    c0 = int(offB[gblocks[0]]) // 16
                    for t0 in range(0, lb, step):
                        tn = min(step, lb - t0)
                        nc.gpsimd.dma_gather(
                            mB[:, t0:t0 + tn, :], tabB,
                            idx_t[:, c0 + t0 * 8:c0 + (t0 + tn) * 8],
                            num_idxs=tn * 128, num_idxs_reg=tn * 128,
                            elem_size=H, elem_step=H)
                msgs[g] = (mA, mB, gblocks[0])

            issue_group(0)
            for si, sblocks in enumerate(supers):
                w = len(sblocks)
                ps = psagg.tile([128, w, H], f32, tag="agg")
                for bi, b in enumerate(sblocks):
                    g = group_of[b]
                    if g not in msgs:
                        issue_group(g)
                    if g + 1 < len(groups) and (g + 1) not in msgs \
                            and b == groups[g][-1]:
                        issue_group(g + 1)
                    mA, mB, b0 = msgs[g]
                    jA = int(offA[b] - offA[b0]) // 128
                    jB0 = int(offB[b] - offA[b0]) // 128
                    la_g = int(sum(D_A[bb] for bb in groups[g]))
                    jB = int(offB[b] - offB[groups[g][0]]) // 128
                    chain = [(mA, jA + j) for j in range(int(D_A[b]))]
                    chain += [(mB, jB + j) for j in range(int(D_B[b]))]
                    nchain = len(chain)
                    for ci, (mt, jj) in enumerate(chain):
                        nc.tensor.matmul(
                            ps[:, bi, :],
                            ddiag_t[:, b * 128:(b + 1) * 128],
                            mt[:, jj, :],
                            start=(ci == 0), stop=(ci == nchain - 1))

                bsl = slice(sblocks[0], sblocks[0] + w)
                t = work.tile([128, w, H], f32, tag="ep")
                nc.vector.tensor_add(t[:], ps[:], brep_t[:].rearrange(
                    "p (l s c) -> p l s c", l=L, s=4)[:, l, :w, :])
                if l == 0:
                    nc.scalar.activation(h_store[:, bsl, :], t[:], RELU)
                else:
                    tmp = work.tile([128, w, H], f32, tag="rel")
                    nc.scalar.activation(tmp[:], t[:], RELU)
                    nc.vector.tensor_add(h_store[:, bsl, :], tmp[:],
                                         h_store[:, bsl, :])
                hb = work.tile([128, w, H], bf16, tag="hb")
                nc.vector.tensor_copy(hb[:], h_store[:, bsl, :])

                for s in range(w):
                    b = sblocks[0] + s
                    if l < L - 1:
                        pst = psw.tile([128, H], f32, tag="pT")
                        nc.tensor.matmul(pst[:], hb[:, s, :],
                                         ddiag_t[:, b * 128:(b + 1) * 128],
                                         start=True, stop=True)
                        hT = work.tile([128, H], bf16, tag="hT")
                        nc.vector.tensor_copy(hT[:], pst[:])
                        ps2 = psw.tile([128, H], f32, tag="p2")
                        nc.tensor.matmul(ps2[:], hT[:],
                                         wc_t[:, (l + 1) * H:(l + 2) * H],
                                         start=True, stop=True)
                        hw_t = work.tile([128, H], bf16, tag="hwt")
                        nc.vector.tensor_copy(hw_t[:], ps2[:])
                        nc.sync.dma_start(hwb[b * 128:(b + 1) * 128, :],
                                          hw_t[:])
                    else:
                        # final MLP on this block
                        pst = psw.tile([128, H], f32, tag="pT")
                        nc.tensor.matmul(pst[:], hb[:, s, :], ident_t[:],
                                         start=True, stop=True)
                        hT = work.tile([128, H], bf16, tag="hT")
                        nc.vector.tensor_copy(hT[:], pst[:])
                        psm = psw.tile([H // 2, H], f32, tag="p2")
                        nc.tensor.matmul(psm[:], w1_t[:], hT[:],
                                         start=True, stop=True)
                        m1 = work.tile([H // 2, H], bf16, tag="m1")
                        nc.scalar.activation(m1[:], psm[:], RELU,
                                             bias=b1_t[:])
                        pso = psw.tile([1, H], f32, tag="pT")
                        nc.tensor.matmul(pso[:], w2_t[:], m1[:],
                                         start=True, stop=True)
                        oseg = work.tile([1, H], f32, tag="oseg")
                        nc.vector.tensor_scalar_add(oseg[:], pso[:], b2f)
                        nc.sync.dma_start(out_d[b:b + 1, :], oseg[:])
                # release consumed groups (tiles rotate out via pool bufs)

            if l < L - 1:
                allgather(tabs[l + 1])

    nc.compile()
    return nc


# ============================ numpy emulation (debug) ============================

def _emulate(x, edge_index, W_in, b_in, Wc, bc, gamma, beta, rmean, rvar,
             W1, b1, W2, b2):
    """Mirror of the device dataflow in numpy (with bf16 rounding at the same
    points). Validates preprocessing + factorized algebra without hardware."""
    import scipy.sparse as sp
    meta, per_core, shared, b2f = _prep(
        x, edge_index, W_in, b_in, Wc, bc, gamma, beta, rmean, rvar,
        W1, b1, W2, b2)
    slot_of, node_of = meta["slot_of"], meta["node_of"]

    def q(a):
        return a.astype(BF16).astype(np.float32)

    ei = np.asarray(edge_index, np.int64)
    src = np.concatenate([ei[0], np.arange(N, dtype=np.int64)])
    dst = np.concatenate([ei[1], np.arange(N, dtype=np.int64)])
    gs, gd = slot_of[src], slot_of[dst]
    A = sp.csr_matrix((np.ones(gs.size, np.float32), (gd, gs)),
                      shape=(NTOT, NTOT))

    deg = np.bincount(dst, minlength=N)
    dis = (1.0 / np.sqrt(deg.astype(np.float64))).astype(np.float32)
    dis_slot = np.zeros(NTOT, np.float32)
    dis_slot[slot_of] = dis
    dis_b = q(dis_slot)

    xp = np.zeros((NTOT, DIN), np.float32)
    xp[slot_of] = np.asarray(x, np.float32)

    rs = 1.0 / np.sqrt(np.asarray(rvar, np.float64) + EPS)
    a_l = (rs * np.asarray(gamma, np.float64)).astype(np.float32)
    b_l = ((np.asarray(bc, np.float64) - np.asarray(rmean, np.float64))
           * rs * np.asarray(gamma, np.float64)
           + np.asarray(beta, np.float64)).astype(np.float32)
    wcp = np.asarray(Wc, np.float32) * a_l[:, None, :]

    h0 = np.maximum(q(xp) @ q(np.asarray(W_in, np.float32)) + b_in, 0)
    tab = q(q(dis_b[:, None] * q(h0)) @ q(wcp[0]))
    h = None
    for l in range(L):
        aggr = A @ tab                       # sum of bf16 msgs, f32 accum
        t = dis_b[:, None] * aggr + b_l[l]
        hn = np.maximum(t, 0)
        h = hn if l == 0 else hn + h
        if l < L - 1:
            hd = q(q(h) * dis_b[:, None])
            tab = q(hd @ q(wcp[l + 1]))
    m1 = np.maximum(q(q(h)) @ q(np.asarray(W1, np.float32))
                    + np.asarray(b1, np.float32), 0)
    o = (q(m1) @ q(np.asarray(W2, np.float32))).reshape(-1) + b2f
    out = np.empty(N, np.float32)
    out[node_of[slot_of]] = o[slot_of]
    return out


# ============================ entry point ============================

def kernel(x, edge_index, W_in, b_in, Wc, bc, gamma, beta, rmean, rvar,
           W1, b1, W2, b2):
    global LAST_EXEC_NS, TRACE_DIR
    from concourse.bass_utils import run_bass_kernel_spmd

    meta, per_core, shared, b2f = _prep(
        x, edge_index, W_in, b_in, Wc, bc, gamma, beta, rmean, rvar,
        W1, b1, W2, b2)

    ck = ("k1", tuple(meta["D_A"]), tuple(meta["D_B"]), b2f)
    if ck not in _CACHE:
        _CACHE.clear()
        _CACHE[ck] = _build(meta, b2f)
    nc = _CACHE[ck]

    in_maps = []
    for c in range(NCORES):
        m = dict(per_core[c])
        m.update(shared)
        in_maps.append(m)

    kwargs = {}
    if TRACE:
        _install_ntff_hook()
        kwargs = dict(trace=True)
    res = run_bass_kernel_spmd(nc, in_maps, list(range(NCORES)), **kwargs)
    LAST_EXEC_NS = res.exec_time_ns
    if TRACE and res.instructions_and_trace is not None:
        TRACE_DIR = res.instructions_and_trace[1]

    node_of, slot_of = meta["node_of"], meta["slot_of"]
    out = np.empty(N, np.float32)
    for c in range(NCORES):
        o = np.asarray(res.results[c]["out"]).reshape(-1)
        sl = np.arange(c * NPC, (c + 1) * NPC)
        real = node_of[sl] >= 0
        out[node_of[sl[real]]] = o[real.nonzero()[0]]
    return out


def _install_ntff_hook():
    import sys, types
    try:
        import antenv.axon_hooks  # noqa: F401
        return
    except ImportError:
        pass
    from trn_agent_boot.trn_boot import _ntff_profile_via_ctypes
    mod = types.ModuleType("antenv.axon_hooks")
    _h = [None]
    mod.set_axon_ntff_profile_hook = lambda h: _h.__setitem__(0, h)
    mod.get_axon_ntff_profile_hook = lambda: _h[0]
    sys.modules["antenv.axon_hooks"] = mod
    import antenv
    antenv.axon_hooks = mod
    mod.set_axon_ntff_profile_hook(
        _ntff_profile_via_ctypes("/opt/axon/libaxon_pjrt.so"))


# revision 12
# speedup vs baseline: 1.0680x; 1.0680x over previous
"""Trainium2 Bass kernel for AtomExposureGNN (3-layer GCN, N=50000, E=800000).

Distribution: nodes are partitioned across 8 NeuronCores (graph parallel).
Host-side (free, untimed): self-loops, degrees, symmetric-norm factorization,
degree-balanced node permutation, per-dst-block padded neighbor index lists.
Device: per layer, each core gathers message rows hw[src] (dma_gather from its
DRAM copy of the table), accumulates them per 128-node dst block on the
TensorEngine (PSUM, with dis[dst] folded into the stationary diag operand),
applies the folded batchnorm affine + relu + residual, computes the next
table shard hw = (dis*h) @ Wc', and AllGathers shards into the next table.

Key algebra: with dis = deg^-1/2,
  agg[d] = sum_e dis[s]dis[d] (h Wc)[s] = dis[d] * sum_e ((dis*h) Wc)[s]
  bn(agg+bc) = agg*a + b  with a,b per-channel  ->  a folds into Wc columns.
"""

import numpy as np
import ml_dtypes

# ---- problem constants (hardcoded per harness contract) ----
N, E, DIN, H, L = 50000, 800000, 64, 128, 3
NCORES = 8
NPC = 6272            # nodes per core (49 * 128), >= ceil(50000/8)=6250
NB = NPC // 128       # 49 dst blocks per core
NTOT = NPC * NCORES   # 50176 global slots
HALF = NTOT // 2      # region A = slots [0, HALF) (cores 0-3), B = rest
ZTOK = 6250           # zero-token: first pad slot of core 0 / core 4 (local id)
EPS = 1e-5
TILE_CAP = 112        # max message tiles (128 tokens each) per gather group
BF16 = ml_dtypes.bfloat16

LAST_EXEC_NS = None
TRACE = False
TRACE_DIR = None
DEBUG_NO_COLLECTIVE = False   # replace AllGather with local DMA (wrong numerics)
GATHER_MAX_IDX = 32768        # split dma_gather calls to at most this many idxs

_CACHE = {}


# ============================ host preprocessing ============================

def _prep(x, edge_index, W_in, b_in, Wc, bc, gamma, beta, rmean, rvar,
          W1, b1, W2, b2):
    x = np.asarray(x, np.float32)
    ei = np.asarray(edge_index, np.int64)
    src = np.concatenate([ei[0], np.arange(N, dtype=np.int64)])
    dst = np.concatenate([ei[1], np.arange(N, dtype=np.int64)])

    deg = np.bincount(dst, minlength=N)
    dis = (1.0 / np.sqrt(deg.astype(np.float64))).astype(np.float32)

    # node -> core: stripe the degree-sorted order for balance
    order = np.argsort(deg, kind="stable")
    core_of = np.empty(N, np.int64)
    core_of[order] = np.arange(N) % NCORES
    region_a = core_of < (NCORES // 2)

    # per-node count of in-edges whose src lands in region A
    a_cnt = np.bincount(dst[region_a[src]], minlength=N)

    # per-core local slot order: (deg, a_cnt) lexsort keeps per-block padding tight
    slot_of = np.empty(N, np.int64)
    for c in range(NCORES):
        nodes_c = np.flatnonzero(core_of == c)
        o = np.lexsort((a_cnt[nodes_c], deg[nodes_c]))
        slot_of[nodes_c[o]] = c * NPC + np.arange(nodes_c.size)
    node_of = np.full(NTOT, -1, np.int64)
    node_of[slot_of] = np.arange(N)

    gd = slot_of[dst]
    gs = slot_of[src]
    reg = (gs >= HALF).astype(np.int64)
    tok = gs - reg * HALF
    core_e = gd // NPC
    blk = (gd % NPC) // 128
    par = gd % 128

    key = ((core_e * NB + blk) * 2 + reg) * 128 + par
    cnt = np.bincount(key, minlength=NCORES * NB * 2 * 128)
    cnt = cnt.reshape(NCORES, NB, 2, 128)
    D_A = cnt[:, :, 0, :].max(axis=(0, 2)).astype(np.int64)   # [NB]
    D_B = cnt[:, :, 1, :].max(axis=(0, 2)).astype(np.int64)

    # contiguous greedy grouping of blocks for batched gathers
    dsum = D_A + D_B
    groups, cur, acc = [], [], 0
    for b in range(NB):
        if cur and acc + dsum[b] > TILE_CAP:
            groups.append(cur)
            cur, acc = [], 0
        cur.append(b)
        acc += int(dsum[b])
    groups.append(cur)

    # idx sequence layout: per group, A spans of its blocks then B spans
    offA = np.zeros(NB, np.int64)
    offB = np.zeros(NB, np.int64)
    off = 0
    for gblocks in groups:
        for b in gblocks:
            offA[b] = off
            off += D_A[b] * 128
        for b in gblocks:
            offB[b] = off
            off += D_B[b] * 128
    ltot = int(off)
    assert ltot % 128 == 0

    # fill token slots: edge with within-(core,blk,reg,par) rank j goes to
    # sequence position off{A,B}[blk] + j*128 + par
    ordk = np.argsort(key, kind="stable")
    ks = key[ordk]
    newrun = np.r_[True, ks[1:] != ks[:-1]]
    runstart = np.flatnonzero(newrun)
    runid = np.cumsum(newrun) - 1
    cc = np.arange(ks.size) - runstart[runid]

    idx_all = np.full((NCORES, ltot), ZTOK, np.int16)
    pos = np.where(reg[ordk] == 0, offA[blk[ordk]], offB[blk[ordk]])
    pos = pos + cc * 128 + par[ordk]
    idx_all[core_e[ordk], pos] = tok[ordk].astype(np.int16)

    idx_sb = np.empty((NCORES, 128, ltot // 16), np.int16)
    for c in range(NCORES):
        w = idx_all[c].reshape(ltot // 16, 16).T
        idx_sb[c] = np.tile(w, (8, 1))

    # per-slot dis / x
    dis_slot = np.zeros(NTOT, np.float32)
    dis_slot[slot_of] = dis
    xp = np.zeros((NTOT, DIN), np.float32)
    xp[slot_of] = x

    dcol = np.empty((NCORES, 128, NB), np.float32)
    ddiag = np.zeros((NCORES, 128, NB, 128), np.float32)
    xT = np.empty((NCORES, DIN, NPC), np.float32)
    p_i = np.arange(128)
    for c in range(NCORES):
        d = dis_slot[c * NPC:(c + 1) * NPC].reshape(NB, 128)
        dcol[c] = d.T
        ddiag[c][p_i[:, None], np.arange(NB)[None, :], p_i[:, None]] = d.T
        xT[c] = xp[c * NPC:(c + 1) * NPC].T
    ddiag = ddiag.reshape(NCORES, 128, NB * 128).astype(BF16)

    # folded batchnorm affine
    rs = 1.0 / np.sqrt(np.asarray(rvar, np.float64) + EPS)
    a_l = (rs * np.asarray(gamma, np.float64)).astype(np.float32)       # [L,H]
    b_l = ((np.asarray(bc, np.float64) - np.asarray(rmean, np.float64))
           * rs * np.asarray(gamma, np.float64)
           + np.asarray(beta, np.float64)).astype(np.float32)           # [L,H]
    wcp = (np.asarray(Wc, np.float32) * a_l[:, None, :])                # [L,H,H]
    wc_sb = np.concatenate([wcp[i] for i in range(L)], axis=1).astype(BF16)
    brep = np.tile(b_l[None, :, None, :], (128, 1, 4, 1))
    brep = brep.reshape(128, L * 4 * H).astype(np.float32)

    assert (D_A + D_B >= 1).all()
    meta = dict(D_A=D_A, D_B=D_B, groups=groups, offA=offA, offB=offB,
                ltot=ltot, node_of=node_of, slot_of=slot_of)
    per_core = []
    for c in range(NCORES):
        per_core.append(dict(
            xT=np.ascontiguousarray(xT[c]),
            idx=np.ascontiguousarray(idx_sb[c]),
            ddiag=np.ascontiguousarray(ddiag[c]),
            dcol=np.ascontiguousarray(dcol[c]),
        ))
    shared = dict(
        win=np.asarray(W_in, np.float32).astype(BF16),                  # [64,128]
        wc=wc_sb,                                                       # [128,3H]
        w1=np.asarray(W1, np.float32).astype(BF16),                     # [128,64]
        w2=np.asarray(W2, np.float32).astype(BF16),                     # [64,1]
        bin=np.asarray(b_in, np.float32).reshape(H, 1),
        b1=np.asarray(b1, np.float32).reshape(H // 2, 1),
        brep=brep,
        ident=np.eye(128, dtype=np.float32).astype(BF16),
    )
    b2f = float(np.asarray(b2, np.float32).reshape(-1)[0])
    return meta, per_core, shared, b2f


# ============================ device kernel ============================

def _build(meta, b2f):
    import concourse.bass as bass
    import concourse.bacc as bacc
    import concourse.tile as tile
    import concourse.mybir as mybir
    from contextlib import ExitStack

    f32 = mybir.dt.float32
    bf16 = mybir.dt.bfloat16
    i16 = mybir.dt.int16
    RELU = mybir.ActivationFunctionType.Relu

    D_A, D_B = meta["D_A"], meta["D_B"]
    groups = meta["groups"]
    offA, offB = meta["offA"], meta["offB"]
    ltot = meta["ltot"]

    group_of = {}
    for g, gblocks in enumerate(groups):
        for b in gblocks:
            group_of[b] = g

    nc = bacc.Bacc("TRN2", target_bir_lowering=False, debug=False,
                   num_devices=NCORES)

    xT_d = nc.dram_tensor("xT", [DIN, NPC], f32, kind="ExternalInput")
    idx_d = nc.dram_tensor("idx", [128, ltot // 16], i16, kind="ExternalInput")
    ddiag_d = nc.dram_tensor("ddiag", [128, NB * 128], bf16, kind="ExternalInput")
    dcol_d = nc.dram_tensor("dcol", [128, NB], f32, kind="ExternalInput")
    win_d = nc.dram_tensor("win", [DIN, H], bf16, kind="ExternalInput")
    wc_d = nc.dram_tensor("wc", [H, L * H], bf16, kind="ExternalInput")
    w1_d = nc.dram_tensor("w1", [H, H // 2], bf16, kind="ExternalInput")
    w2_d = nc.dram_tensor("w2", [H // 2, 1], bf16, kind="ExternalInput")
    bin_d = nc.dram_tensor("bin", [H, 1], f32, kind="ExternalInput")
    b1_d = nc.dram_tensor("b1", [H // 2, 1], f32, kind="ExternalInput")
    brep_d = nc.dram_tensor("brep", [128, L * 4 * H], f32, kind="ExternalInput")
    ident_d = nc.dram_tensor("ident", [128, 128], bf16, kind="ExternalInput")
    out_d = nc.dram_tensor("out", [NB, 128], f32, kind="ExternalOutput")

    hwb = nc.dram_tensor("hwb", [NPC, H], bf16)
    tabs = [nc.dram_tensor(f"tab{i}", [NTOT, H], bf16, addr_space="Shared")
            for i in range(L)]
    rg = [list(range(NCORES))]

    with tile.TileContext(nc) as tc, ExitStack() as ctx:
        const = ctx.enter_context(tc.tile_pool(name="const", bufs=1))
        hpool = ctx.enter_context(tc.tile_pool(name="hst", bufs=1))
        msgp = ctx.enter_context(tc.tile_pool(name="msg", bufs=2))
        work = ctx.enter_context(tc.tile_pool(name="work", bufs=3))
        psagg = ctx.enter_context(
            tc.tile_pool(name="psagg", bufs=2, space="PSUM"))
        psw = ctx.enter_context(tc.tile_pool(name="psw", bufs=2, space="PSUM"))

        # ---- persistent SBUF residents ----
        idx_t = const.tile([128, ltot // 16], i16)
        nc.sync.dma_start(idx_t[:], idx_d[:])
        ddiag_t = const.tile([128, NB * 128], bf16)
        nc.sync.dma_start(ddiag_t[:], ddiag_d[:])
        dcol_t = const.tile([128, NB], f32)
        nc.sync.dma_start(dcol_t[:], dcol_d[:])
        brep_t = const.tile([128, L * 4 * H], f32)
        nc.sync.dma_start(brep_t[:], brep_d[:])
        win_t = const.tile([DIN, H], bf16)
        nc.sync.dma_start(win_t[:], win_d[:])
        wc_t = const.tile([H, L * H], bf16)
        nc.sync.dma_start(wc_t[:], wc_d[:])
        w1_t = const.tile([H, H // 2], bf16)
        nc.sync.dma_start(w1_t[:], w1_d[:])
        w2_t = const.tile([H // 2, 1], bf16)
        nc.sync.dma_start(w2_t[:], w2_d[:])
        bin_t = const.tile([H, 1], f32)
        nc.sync.dma_start(bin_t[:], bin_d[:])
        b1_t = const.tile([H // 2, 1], f32)
        nc.sync.dma_start(b1_t[:], b1_d[:])
        ident_t = const.tile([128, 128], bf16)
        nc.sync.dma_start(ident_t[:], ident_d[:])

        h_store = hpool.tile([128, NB, H], f32)

        # ---- layer 0: hw0 = dis * (relu(x W_in + b_in) @ Wc'0), own shard ----
        done = 0
        l0chunks = [512] * (NPC // 512)
        if NPC % 512:
            l0chunks.append(NPC % 512)
        for w in l0chunks:
            sl = slice(done, done + w)
            xt_b = work.tile([DIN, w], bf16, tag="xt")
            nc.gpsimd.dma_start(xt_b[:], xT_d[:, sl])       # f32 -> bf16 cast
            ps0 = psw.tile([H, w], f32, tag="pT")
            nc.tensor.matmul(ps0[:], win_t[:], xt_b[:], start=True, stop=True)
            h0 = work.tile([H, w], bf16, tag="h0")
            nc.scalar.activation(h0[:], ps0[:], RELU, bias=bin_t[:])
            for s in range(w // 128):
                b = done // 128 + s
                ps2 = psw.tile([128, H], f32, tag="p2")
                nc.tensor.matmul(ps2[:], h0[:, s * 128:(s + 1) * 128],
                                 wc_t[:, 0:H], start=True, stop=True)
                hw_t = work.tile([128, H], bf16, tag="hwt")
                nc.vector.tensor_scalar_mul(hw_t[:], ps2[:],
                                            dcol_t[:, b:b + 1])
                nc.sync.dma_start(hwb[b * 128:(b + 1) * 128, :], hw_t[:])
            done += w

        def allgather(dst):
            if DEBUG_NO_COLLECTIVE:
                nc.gpsimd.dma_start(dst[0:NPC, :], hwb[:])
            else:
                nc.gpsimd.collective_compute(
                    "AllGather", mybir.AluOpType.bypass, replica_groups=rg,
                    ins=[hwb[:]], outs=[dst[:]])

        allgather(tabs[0])

        # ---- GCN layers ----
        supers = [list(range(s, min(s + 4, NB))) for s in range(0, NB, 4)]

        for l in range(L):
            tabA = tabs[l][:HALF, :]
            tabB = tabs[l][HALF:, :]

            msgs = {}

            def issue_group(g):
                gblocks = groups[g]
                la = int(sum(D_A[b] for b in gblocks))
                lb = int(sum(D_B[b] for b in gblocks))
                mA = mB = None
                step = max(1, GATHER_MAX_IDX // 128)
                if la:
                    mA = msgp.tile([128, la, H], bf16, tag="mA")
                    c0 = int(offA[gblocks[0]]) // 16
                    for t0 in range(0, la, step):
                        tn = min(step, la - t0)
                        nc.gpsimd.dma_gather(
                            mA[:, t0:t0 + tn, :], tabA,
                            idx_t[:, c0 + t0 * 8:c0 + (t0 + tn) * 8],
                            num_idxs=tn * 128, num_idxs_reg=tn * 128,
                            elem_size=H, elem_step=H, single_packet=False)
                if lb:
                    mB = msgp.tile([128, lb, H], bf16, tag="mB")
                    c0 = int(offB[gblocks[0]]) // 16
                    for t0 in range(0, lb, step):
                        tn = min(step, lb - t0)
                        nc.gpsimd.dma_gather(
                            mB[:, t0:t0 + tn, :], tabB,
                            idx_t[:, c0 + t0 * 8:c0 + (t0 + tn) * 8],
                            num_idxs=tn * 128, num_idxs_reg=tn * 128,
                            elem_size=H, elem_step=H, single_packet=False)
                msgs[g] = (mA, mB, gblocks[0])

            issue_group(0)
            for si, sblocks in enumerate(supers):
                w = len(sblocks)
                ps = psagg.tile([128, w, H], f32, tag="agg")
                for bi, b in enumerate(sblocks):
                    g = group_of[b]
                    if g not in msgs:
                        issue_group(g)
                    if g + 1 < len(groups) and (g + 1) not in msgs \
                            and b == groups[g][-1]:
                        issue_group(g + 1)
                    mA, mB, b0 = msgs[g]
                    jA = int(offA[b] - offA[b0]) // 128
                    jB0 = int(offB[b] - offA[b0]) // 128
                    la_g = int(sum(D_A[bb] for bb in groups[g]))
                    jB = int(offB[b] - offB[groups[g][0]]) // 128
                    chain = [(mA, jA + j) for j in range(int(D_A[b]))]
                    chain += [(mB, jB + j) for j in range(int(D_B[b]))]
                    nchain = len(chain)
                    for ci, (mt, jj) in enumerate(chain):
                        nc.tensor.matmul(
                            ps[:, bi, :],
                            ddiag_t[:, b * 128:(b + 1) * 128],
                            mt[:, jj, :],
                            start=(ci == 0), stop=(ci == nchain - 1))

                bsl = slice(sblocks[0], sblocks[0] + w)
                t = work.tile([128, w, H], f32, tag="ep")
                nc.vector.tensor_add(t[:], ps[:], brep_t[:].rearrange(
                    "p (l s c) -> p l s c", l=L, s=4)[:, l, :w, :])
                if l == 0:
                    nc.scalar.activation(h_store[:, bsl, :], t[:], RELU)
                else:
                    tmp = work.tile([128, w, H], f32, tag="rel")
                    nc.scalar.activation(tmp[:], t[:], RELU)
                    nc.vector.tensor_add(h_store[:, bsl, :], tmp[:],
                                         h_store[:, bsl, :])
                hb = work.tile([128, w, H], bf16, tag="hb")
                nc.vector.tensor_copy(hb[:], h_store[:, bsl, :])

                for s in range(w):
                    b = sblocks[0] + s
                    if l < L - 1:
                        pst = psw.tile([128, H], f32, tag="pT")
                        nc.tensor.matmul(pst[:], hb[:, s, :],
                                         ddiag_t[:, b * 128:(b + 1) * 128],
                                         start=True, stop=True)
                        hT = work.tile([128, H], bf16, tag="hT")
                        nc.vector.tensor_copy(hT[:], pst[:])
                        ps2 = psw.tile([128, H], f32, tag="p2")
                        nc.tensor.matmul(ps2[:], hT[:],
                                         wc_t[:, (l + 1) * H:(l + 2) * H],
                                         start=True, stop=True)
                        hw_t = work.tile([128, H], bf16, tag="hwt")
                        nc.vector.tensor_copy(hw_t[:], ps2[:])
                        nc.sync.dma_start(hwb[b * 128:(b + 1) * 128, :],
                                          hw_t[:])
                    else:
                        # final MLP on this block
                        pst = psw.tile([128, H], f32, tag="pT")
                        nc.tensor.matmul(pst[:], hb[:, s, :], ident_t[:],
                                         start=True, stop=True)
                        hT = work.tile([128, H], bf16, tag="hT")
                        nc.vector.tensor_copy(hT[:], pst[:])
                        psm = psw.tile([H // 2, H], f32, tag="p2")
                        nc.tensor.matmul(psm[:], w1_t[:], hT[:],
                                         start=True, stop=True)
                        m1 = work.tile([H // 2, H], bf16, tag="m1")
                        nc.scalar.activation(m1[:], psm[:], RELU,
                                             bias=b1_t[:])
                        pso = psw.tile([1, H], f32, tag="pT")
                        nc.tensor.matmul(pso[:], w2_t[:], m1[:],
                                         start=True, stop=True)
                        oseg = work.tile([1, H], f32, tag="oseg")
                        nc.vector.tensor_scalar_add(oseg[:], pso[:], b2f)
                        nc.sync.dma_start(out_d[b:b + 1, :], oseg[:])
                # release consumed groups (tiles rotate out via pool bufs)

            if l < L - 1:
                allgather(tabs[l + 1])

    nc.compile()
    return nc


# ============================ numpy emulation (debug) ============================

def _emulate(x, edge_index, W_in, b_in, Wc, bc, gamma, beta, rmean, rvar,
             W1, b1, W2, b2):
    """Mirror of the device dataflow in numpy (with bf16 rounding at the same
    points). Validates preprocessing + factorized algebra without hardware."""
    import scipy.sparse as sp
    meta, per_core, shared, b2f = _prep(
        x, edge_index, W_in, b_in, Wc, bc, gamma, beta, rmean, rvar,
        W1, b1, W2, b2)
    slot_of, node_of = meta["slot_of"], meta["node_of"]

    def q(a):
        return a.astype(BF16).astype(np.float32)

    ei = np.asarray(edge_index, np.int64)
    src = np.concatenate([ei[0], np.arange(N, dtype=np.int64)])
    dst = np.concatenate([ei[1], np.arange(N, dtype=np.int64)])
    gs, gd = slot_of[src], slot_of[dst]
    A = sp.csr_matrix((np.ones(gs.size, np.float32), (gd, gs)),
                      shape=(NTOT, NTOT))

    deg = np.bincount(dst, minlength=N)
    dis = (1.0 / np.sqrt(deg.astype(np.float64))).astype(np.float32)
    dis_slot = np.zeros(NTOT, np.float32)
    dis_slot[slot_of] = dis
    dis_b = q(dis_slot)

    xp = np.zeros((NTOT, DIN), np.float32)
    xp[slot_of] = np.asarray(x, np.float32)

    rs = 1.0 / np.sqrt(np.asarray(rvar, np.float64) + EPS)
    a_l = (rs * np.asarray(gamma, np.float64)).astype(np.float32)
    b_l = ((np.asarray(bc, np.float64) - np.asarray(rmean, np.float64))
           * rs * np.asarray(gamma, np.float64)
           + np.asarray(beta, np.float64)).astype(np.float32)
    wcp = np.asarray(Wc, np.float32) * a_l[:, None, :]

    h0 = np.maximum(q(xp) @ q(np.asarray(W_in, np.float32)) + b_in, 0)
    tab = q(q(dis_b[:, None] * q(h0)) @ q(wcp[0]))
    h = None
    for l in range(L):
        aggr = A @ tab                       # sum of bf16 msgs, f32 accum
        t = dis_b[:, None] * aggr + b_l[l]
        hn = np.maximum(t, 0)
        h = hn if l == 0 else hn + h
        if l < L - 1:
            hd = q(q(h) * dis_b[:, None])
            tab = q(hd @ q(wcp[l + 1]))
    m1 = np.maximum(q(q(h)) @ q(np.asarray(W1, np.float32))
                    + np.asarray(b1, np.float32), 0)
    o = (q(m1) @ q(np.asarray(W2, np.float32))).reshape(-1) + b2f
    out = np.empty(N, np.float32)
    out[node_of[slot_of]] = o[slot_of]
    return out


# ============================ entry point ============================

def kernel(x, edge_index, W_in, b_in, Wc, bc, gamma, beta, rmean, rvar,
           W1, b1, W2, b2):
    global LAST_EXEC_NS, TRACE_DIR
    from concourse.bass_utils import run_bass_kernel_spmd

    meta, per_core, shared, b2f = _prep(
        x, edge_index, W_in, b_in, Wc, bc, gamma, beta, rmean, rvar,
        W1, b1, W2, b2)

    ck = ("k1", tuple(meta["D_A"]), tuple(meta["D_B"]), b2f)
    if ck not in _CACHE:
        _CACHE.clear()
        _CACHE[ck] = _build(meta, b2f)
    nc = _CACHE[ck]

    in_maps = []
    for c in range(NCORES):
        m = dict(per_core[c])
        m.update(shared)
        in_maps.append(m)

    kwargs = {}
    if TRACE:
        _install_ntff_hook()
        kwargs = dict(trace=True)
    res = run_bass_kernel_spmd(nc, in_maps, list(range(NCORES)), **kwargs)
    LAST_EXEC_NS = res.exec_time_ns
    if TRACE and res.instructions_and_trace is not None:
        TRACE_DIR = res.instructions_and_trace[1]

    node_of, slot_of = meta["node_of"], meta["slot_of"]
    out = np.empty(N, np.float32)
    for c in range(NCORES):
        o = np.asarray(res.results[c]["out"]).reshape(-1)
        sl = np.arange(c * NPC, (c + 1) * NPC)
        real = node_of[sl] >= 0
        out[node_of[sl[real]]] = o[real.nonzero()[0]]
    return out


def _install_ntff_hook():
    import sys, types
    try:
        import antenv.axon_hooks  # noqa: F401
        return
    except ImportError:
        pass
    from trn_agent_boot.trn_boot import _ntff_profile_via_ctypes
    mod = types.ModuleType("antenv.axon_hooks")
    _h = [None]
    mod.set_axon_ntff_profile_hook = lambda h: _h.__setitem__(0, h)
    mod.get_axon_ntff_profile_hook = lambda: _h[0]
    sys.modules["antenv.axon_hooks"] = mod
    import antenv
    antenv.axon_hooks = mod
    mod.set_axon_ntff_profile_hook(
        _ntff_profile_via_ctypes("/opt/axon/libaxon_pjrt.so"))


# revision 17
# speedup vs baseline: 1.4447x; 1.3527x over previous
"""Trainium2 Bass kernel for AtomExposureGNN (3-layer GCN, N=50000, E=800000).

Distribution: nodes are partitioned across 8 NeuronCores (graph parallel).
Host-side (free, untimed): self-loops, degrees, symmetric-norm factorization,
degree-balanced node permutation, per-dst-block padded neighbor index lists.
Device: per layer, each core gathers message rows hw[src] (dma_gather from its
DRAM copy of the table), accumulates them per 128-node dst block on the
TensorEngine (PSUM, with dis[dst] folded into the stationary diag operand),
applies the folded batchnorm affine + relu + residual, computes the next
table shard hw = (dis*h) @ Wc', and AllGathers shards into the next table.

Key algebra: with dis = deg^-1/2,
  agg[d] = sum_e dis[s]dis[d] (h Wc)[s] = dis[d] * sum_e ((dis*h) Wc)[s]
  bn(agg+bc) = agg*a + b  with a,b per-channel  ->  a folds into Wc columns.
"""

import numpy as np
import ml_dtypes

# ---- problem constants (hardcoded per harness contract) ----
N, E, DIN, H, L = 50000, 800000, 64, 128, 3
NCORES = 8
NPC = 6272            # nodes per core (49 * 128), >= ceil(50000/8)=6250
NB = NPC // 128       # 49 dst blocks per core
NTOT = NPC * NCORES   # 50176 global slots
HALF = NTOT // 2      # region A = slots [0, HALF) (cores 0-3), B = rest
ZTOK = 6250           # zero-token: first pad slot of core 0 / core 4 (local id)
EPS = 1e-5
TILE_CAP = 112        # max message tiles (128 tokens each) per gather group
BF16 = ml_dtypes.bfloat16

LAST_EXEC_NS = None
TRACE = False
TRACE_DIR = None
DEBUG_NO_COLLECTIVE = False   # replace AllGather with local DMA (wrong numerics)
GATHER_MAX_IDX = 32768        # split dma_gather calls to at most this many idxs
GATHER_QUEUES = 1             # spread gathers over this many SWDGE queues
MSG_BUFS = 2                  # msg pool double/triple buffering

_CACHE = {}


# ============================ host preprocessing ============================

def _prep(x, edge_index, W_in, b_in, Wc, bc, gamma, beta, rmean, rvar,
          W1, b1, W2, b2):
    x = np.asarray(x, np.float32)
    ei = np.asarray(edge_index, np.int64)
    src = np.concatenate([ei[0], np.arange(N, dtype=np.int64)])
    dst = np.concatenate([ei[1], np.arange(N, dtype=np.int64)])

    deg = np.bincount(dst, minlength=N)
    dis = (1.0 / np.sqrt(deg.astype(np.float64))).astype(np.float32)

    # node -> core: stripe the degree-sorted order for balance
    order = np.argsort(deg, kind="stable")
    core_of = np.empty(N, np.int64)
    core_of[order] = np.arange(N) % NCORES
    region_a = core_of < (NCORES // 2)

    # per-node count of in-edges whose src lands in region A
    a_cnt = np.bincount(dst[region_a[src]], minlength=N)

    # per-core local slot order: (deg, a_cnt) lexsort keeps per-block padding tight
    slot_of = np.empty(N, np.int64)
    for c in range(NCORES):
        nodes_c = np.flatnonzero(core_of == c)
        o = np.lexsort((a_cnt[nodes_c], deg[nodes_c]))
        slot_of[nodes_c[o]] = c * NPC + np.arange(nodes_c.size)
    node_of = np.full(NTOT, -1, np.int64)
    node_of[slot_of] = np.arange(N)

    gd = slot_of[dst]
    gs = slot_of[src]
    reg = (gs >= HALF).astype(np.int64)
    tok = gs - reg * HALF
    core_e = gd // NPC
    blk = (gd % NPC) // 128
    par = gd % 128

    key = ((core_e * NB + blk) * 2 + reg) * 128 + par
    cnt = np.bincount(key, minlength=NCORES * NB * 2 * 128)
    cnt = cnt.reshape(NCORES, NB, 2, 128)
    D_A = cnt[:, :, 0, :].max(axis=(0, 2)).astype(np.int64)   # [NB]
    D_B = cnt[:, :, 1, :].max(axis=(0, 2)).astype(np.int64)

    # contiguous greedy grouping of blocks for batched gathers
    dsum = D_A + D_B
    groups, cur, acc = [], [], 0
    for b in range(NB):
        if cur and acc + dsum[b] > TILE_CAP:
            groups.append(cur)
            cur, acc = [], 0
        cur.append(b)
        acc += int(dsum[b])
    groups.append(cur)

    # idx sequence layout: per group, A spans of its blocks then B spans
    offA = np.zeros(NB, np.int64)
    offB = np.zeros(NB, np.int64)
    off = 0
    for gblocks in groups:
        for b in gblocks:
            offA[b] = off
            off += D_A[b] * 128
        for b in gblocks:
            offB[b] = off
            off += D_B[b] * 128
    ltot = int(off)
    assert ltot % 128 == 0

    # fill token slots: edge with within-(core,blk,reg,par) rank j goes to
    # sequence position off{A,B}[blk] + j*128 + par
    ordk = np.argsort(key, kind="stable")
    ks = key[ordk]
    newrun = np.r_[True, ks[1:] != ks[:-1]]
    runstart = np.flatnonzero(newrun)
    runid = np.cumsum(newrun) - 1
    cc = np.arange(ks.size) - runstart[runid]

    idx_all = np.full((NCORES, ltot), ZTOK, np.int16)
    pos = np.where(reg[ordk] == 0, offA[blk[ordk]], offB[blk[ordk]])
    pos = pos + cc * 128 + par[ordk]
    idx_all[core_e[ordk], pos] = tok[ordk].astype(np.int16)

    idx_sb = np.empty((NCORES, 128, ltot // 16), np.int16)
    for c in range(NCORES):
        w = idx_all[c].reshape(ltot // 16, 16).T
        idx_sb[c] = np.tile(w, (8, 1))

    # per-slot dis / x
    dis_slot = np.zeros(NTOT, np.float32)
    dis_slot[slot_of] = dis
    xp = np.zeros((NTOT, DIN), np.float32)
    xp[slot_of] = x

    dcol = np.empty((NCORES, 128, NB), np.float32)
    ddiag = np.zeros((NCORES, 128, NB, 128), np.float32)
    xT = np.empty((NCORES, DIN, NPC), np.float32)
    p_i = np.arange(128)
    for c in range(NCORES):
        d = dis_slot[c * NPC:(c + 1) * NPC].reshape(NB, 128)
        dcol[c] = d.T
        ddiag[c][p_i[:, None], np.arange(NB)[None, :], p_i[:, None]] = d.T
        xT[c] = xp[c * NPC:(c + 1) * NPC].T
    ddiag = ddiag.reshape(NCORES, 128, NB * 128).astype(BF16)

    # folded batchnorm affine
    rs = 1.0 / np.sqrt(np.asarray(rvar, np.float64) + EPS)
    a_l = (rs * np.asarray(gamma, np.float64)).astype(np.float32)       # [L,H]
    b_l = ((np.asarray(bc, np.float64) - np.asarray(rmean, np.float64))
           * rs * np.asarray(gamma, np.float64)
           + np.asarray(beta, np.float64)).astype(np.float32)           # [L,H]
    wcp = (np.asarray(Wc, np.float32) * a_l[:, None, :])                # [L,H,H]
    wc_sb = np.concatenate([wcp[i] for i in range(L)], axis=1).astype(BF16)
    brep = np.tile(b_l[None, :, None, :], (128, 1, 4, 1))
    brep = brep.reshape(128, L * 4 * H).astype(np.float32)

    assert (D_A + D_B >= 1).all()
    meta = dict(D_A=D_A, D_B=D_B, groups=groups, offA=offA, offB=offB,
                ltot=ltot, node_of=node_of, slot_of=slot_of)
    per_core = []
    for c in range(NCORES):
        per_core.append(dict(
            xT=np.ascontiguousarray(xT[c]),
            idx=np.ascontiguousarray(idx_sb[c]),
            ddiag=np.ascontiguousarray(ddiag[c]),
            dcol=np.ascontiguousarray(dcol[c]),
        ))
    shared = dict(
        win=np.asarray(W_in, np.float32).astype(BF16),                  # [64,128]
        wc=wc_sb,                                                       # [128,3H]
        w1=np.asarray(W1, np.float32).astype(BF16),                     # [128,64]
        w2=np.asarray(W2, np.float32).astype(BF16),                     # [64,1]
        bin=np.asarray(b_in, np.float32).reshape(H, 1),
        b1=np.asarray(b1, np.float32).reshape(H // 2, 1),
        brep=brep,
        ident=np.eye(128, dtype=np.float32).astype(BF16),
    )
    b2f = float(np.asarray(b2, np.float32).reshape(-1)[0])
    return meta, per_core, shared, b2f


# ============================ device kernel ============================

def _build(meta, b2f):
    import concourse.bass as bass
    import concourse.bacc as bacc
    import concourse.tile as tile
    import concourse.mybir as mybir
    from contextlib import ExitStack

    f32 = mybir.dt.float32
    bf16 = mybir.dt.bfloat16
    i16 = mybir.dt.int16
    RELU = mybir.ActivationFunctionType.Relu

    D_A, D_B = meta["D_A"], meta["D_B"]
    groups = meta["groups"]
    offA, offB = meta["offA"], meta["offB"]
    ltot = meta["ltot"]

    group_of = {}
    for g, gblocks in enumerate(groups):
        for b in gblocks:
            group_of[b] = g

    nc = bacc.Bacc("TRN2", target_bir_lowering=False, debug=False,
                   num_devices=NCORES, num_swdge_queues=max(1, GATHER_QUEUES))

    xT_d = nc.dram_tensor("xT", [DIN, NPC], f32, kind="ExternalInput")
    idx_d = nc.dram_tensor("idx", [128, ltot // 16], i16, kind="ExternalInput")
    ddiag_d = nc.dram_tensor("ddiag", [128, NB * 128], bf16, kind="ExternalInput")
    dcol_d = nc.dram_tensor("dcol", [128, NB], f32, kind="ExternalInput")
    win_d = nc.dram_tensor("win", [DIN, H], bf16, kind="ExternalInput")
    wc_d = nc.dram_tensor("wc", [H, L * H], bf16, kind="ExternalInput")
    w1_d = nc.dram_tensor("w1", [H, H // 2], bf16, kind="ExternalInput")
    w2_d = nc.dram_tensor("w2", [H // 2, 1], bf16, kind="ExternalInput")
    bin_d = nc.dram_tensor("bin", [H, 1], f32, kind="ExternalInput")
    b1_d = nc.dram_tensor("b1", [H // 2, 1], f32, kind="ExternalInput")
    brep_d = nc.dram_tensor("brep", [128, L * 4 * H], f32, kind="ExternalInput")
    ident_d = nc.dram_tensor("ident", [128, 128], bf16, kind="ExternalInput")
    out_d = nc.dram_tensor("out", [NB, 128], f32, kind="ExternalOutput")

    hwb = nc.dram_tensor("hwb", [NPC, H], bf16)
    tabs = [nc.dram_tensor(f"tab{i}", [NTOT, H], bf16, addr_space="Shared")
            for i in range(L)]
    rg = [list(range(NCORES))]

    with tile.TileContext(nc) as tc, ExitStack() as ctx:
        const = ctx.enter_context(tc.tile_pool(name="const", bufs=1))
        hpool = ctx.enter_context(tc.tile_pool(name="hst", bufs=1))
        msgp = ctx.enter_context(tc.tile_pool(name="msg", bufs=MSG_BUFS))
        work = ctx.enter_context(tc.tile_pool(name="work", bufs=3))
        psagg = ctx.enter_context(
            tc.tile_pool(name="psagg", bufs=2, space="PSUM"))
        psw = ctx.enter_context(tc.tile_pool(name="psw", bufs=2, space="PSUM"))

        # ---- persistent SBUF residents ----
        idx_t = const.tile([128, ltot // 16], i16)
        nc.sync.dma_start(idx_t[:], idx_d[:])
        ddiag_t = const.tile([128, NB * 128], bf16)
        nc.sync.dma_start(ddiag_t[:], ddiag_d[:])
        dcol_t = const.tile([128, NB], f32)
        nc.sync.dma_start(dcol_t[:], dcol_d[:])
        brep_t = const.tile([128, L * 4 * H], f32)
        nc.sync.dma_start(brep_t[:], brep_d[:])
        win_t = const.tile([DIN, H], bf16)
        nc.sync.dma_start(win_t[:], win_d[:])
        wc_t = const.tile([H, L * H], bf16)
        nc.sync.dma_start(wc_t[:], wc_d[:])
        w1_t = const.tile([H, H // 2], bf16)
        nc.sync.dma_start(w1_t[:], w1_d[:])
        w2_t = const.tile([H // 2, 1], bf16)
        nc.sync.dma_start(w2_t[:], w2_d[:])
        bin_t = const.tile([H, 1], f32)
        nc.sync.dma_start(bin_t[:], bin_d[:])
        b1_t = const.tile([H // 2, 1], f32)
        nc.sync.dma_start(b1_t[:], b1_d[:])
        ident_t = const.tile([128, 128], bf16)
        nc.sync.dma_start(ident_t[:], ident_d[:])

        h_store = hpool.tile([128, NB, H], f32)

        # ---- layer 0: hw0 = dis * (relu(x W_in + b_in) @ Wc'0), own shard ----
        done = 0
        l0chunks = [512] * (NPC // 512)
        if NPC % 512:
            l0chunks.append(NPC % 512)
        for w in l0chunks:
            sl = slice(done, done + w)
            xt_b = work.tile([DIN, w], bf16, tag="xt")
            nc.gpsimd.dma_start(xt_b[:], xT_d[:, sl])       # f32 -> bf16 cast
            ps0 = psw.tile([H, w], f32, tag="pT")
            nc.tensor.matmul(ps0[:], win_t[:], xt_b[:], start=True, stop=True)
            h0 = work.tile([H, w], bf16, tag="h0")
            nc.scalar.activation(h0[:], ps0[:], RELU, bias=bin_t[:])
            for s in range(w // 128):
                b = done // 128 + s
                ps2 = psw.tile([128, H], f32, tag="p2")
                nc.tensor.matmul(ps2[:], h0[:, s * 128:(s + 1) * 128],
                                 wc_t[:, 0:H], start=True, stop=True)
                hw_t = work.tile([128, H], bf16, tag="hwt")
                nc.vector.tensor_scalar_mul(hw_t[:], ps2[:],
                                            dcol_t[:, b:b + 1])
                nc.sync.dma_start(hwb[b * 128:(b + 1) * 128, :], hw_t[:])
            done += w

        def allgather(dst):
            if DEBUG_NO_COLLECTIVE:
                nc.gpsimd.dma_start(dst[0:NPC, :], hwb[:])
            else:
                nc.gpsimd.collective_compute(
                    "AllGather", mybir.AluOpType.bypass, replica_groups=rg,
                    ins=[hwb[:]], outs=[dst[:]])

        allgather(tabs[0])

        # ---- GCN layers ----
        supers = [list(range(s, min(s + 4, NB))) for s in range(0, NB, 4)]
        qn = [0]

        for l in range(L):
            tabA = tabs[l][:HALF, :]
            tabB = tabs[l][HALF:, :]

            msgs = {}

            def issue_group(g):
                gblocks = groups[g]
                la = int(sum(D_A[b] for b in gblocks))
                lb = int(sum(D_B[b] for b in gblocks))
                mA = mB = None
                step = max(1, GATHER_MAX_IDX // 128)
                if la:
                    mA = msgp.tile([128, la, H], bf16, tag="mA")
                    c0 = int(offA[gblocks[0]]) // 16
                    for t0 in range(0, la, step):
                        tn = min(step, la - t0)
                        nc.gpsimd.dma_gather(
                            mA[:, t0:t0 + tn, :], tabA,
                            idx_t[:, c0 + t0 * 8:c0 + (t0 + tn) * 8],
                            num_idxs=tn * 128, num_idxs_reg=tn * 128,
                            elem_size=H, elem_step=H, single_packet=False,
                            queue_num=qn[0] % max(1, GATHER_QUEUES))
                        qn[0] += 1
                if lb:
                    mB = msgp.tile([128, lb, H], bf16, tag="mB")
                    c0 = int(offB[gblocks[0]]) // 16
                    for t0 in range(0, lb, step):
                        tn = min(step, lb - t0)
                        nc.gpsimd.dma_gather(
                            mB[:, t0:t0 + tn, :], tabB,
                            idx_t[:, c0 + t0 * 8:c0 + (t0 + tn) * 8],
                            num_idxs=tn * 128, num_idxs_reg=tn * 128,
                            elem_size=H, elem_step=H, single_packet=False,
                            queue_num=qn[0] % max(1, GATHER_QUEUES))
                        qn[0] += 1
                msgs[g] = (mA, mB, gblocks[0])

            issue_group(0)
            for si, sblocks in enumerate(supers):
                w = len(sblocks)
                ps = psagg.tile([128, w, H], f32, tag="agg")
                for bi, b in enumerate(sblocks):
                    g = group_of[b]
                    if g not in msgs:
                        issue_group(g)
                    if g + 1 < len(groups) and (g + 1) not in msgs \
                            and b == groups[g][-1]:
                        issue_group(g + 1)
                    mA, mB, b0 = msgs[g]
                    jA = int(offA[b] - offA[b0]) // 128
                    jB0 = int(offB[b] - offA[b0]) // 128
                    la_g = int(sum(D_A[bb] for bb in groups[g]))
                    jB = int(offB[b] - offB[groups[g][0]]) // 128
                    chain = [(mA, jA + j) for j in range(int(D_A[b]))]
                    chain += [(mB, jB + j) for j in range(int(D_B[b]))]
                    nchain = len(chain)
                    for ci, (mt, jj) in enumerate(chain):
                        nc.tensor.matmul(
                            ps[:, bi, :],
                            ddiag_t[:, b * 128:(b + 1) * 128],
                            mt[:, jj, :],
                            start=(ci == 0), stop=(ci == nchain - 1))

                bsl = slice(sblocks[0], sblocks[0] + w)
                t = work.tile([128, w, H], f32, tag="ep")
                nc.vector.tensor_add(t[:], ps[:], brep_t[:].rearrange(
                    "p (l s c) -> p l s c", l=L, s=4)[:, l, :w, :])
                if l == 0:
                    nc.scalar.activation(h_store[:, bsl, :], t[:], RELU)
                else:
                    tmp = work.tile([128, w, H], f32, tag="rel")
                    nc.scalar.activation(tmp[:], t[:], RELU)
                    nc.vector.tensor_add(h_store[:, bsl, :], tmp[:],
                                         h_store[:, bsl, :])
                hb = work.tile([128, w, H], bf16, tag="hb")
                nc.vector.tensor_copy(hb[:], h_store[:, bsl, :])

                for s in range(w):
                    b = sblocks[0] + s
                    if l < L - 1:
                        pst = psw.tile([128, H], f32, tag="pT")
                        nc.tensor.matmul(pst[:], hb[:, s, :],
                                         ddiag_t[:, b * 128:(b + 1) * 128],
                                         start=True, stop=True)
                        hT = work.tile([128, H], bf16, tag="hT")
                        nc.vector.tensor_copy(hT[:], pst[:])
                        ps2 = psw.tile([128, H], f32, tag="p2")
                        nc.tensor.matmul(ps2[:], hT[:],
                                         wc_t[:, (l + 1) * H:(l + 2) * H],
                                         start=True, stop=True)
                        hw_t = work.tile([128, H], bf16, tag="hwt")
                        nc.vector.tensor_copy(hw_t[:], ps2[:])
                        nc.sync.dma_start(hwb[b * 128:(b + 1) * 128, :],
                                          hw_t[:])
                    else:
                        # final MLP on this block
                        pst = psw.tile([128, H], f32, tag="pT")
                        nc.tensor.matmul(pst[:], hb[:, s, :], ident_t[:],
                                         start=True, stop=True)
                        hT = work.tile([128, H], bf16, tag="hT")
                        nc.vector.tensor_copy(hT[:], pst[:])
                        psm = psw.tile([H // 2, H], f32, tag="p2")
                        nc.tensor.matmul(psm[:], w1_t[:], hT[:],
                                         start=True, stop=True)
                        m1 = work.tile([H // 2, H], bf16, tag="m1")
                        nc.scalar.activation(m1[:], psm[:], RELU,
                                             bias=b1_t[:])
                        pso = psw.tile([1, H], f32, tag="pT")
                        nc.tensor.matmul(pso[:], w2_t[:], m1[:],
                                         start=True, stop=True)
                        oseg = work.tile([1, H], f32, tag="oseg")
                        nc.vector.tensor_scalar_add(oseg[:], pso[:], b2f)
                        nc.sync.dma_start(out_d[b:b + 1, :], oseg[:])
                # release consumed groups (tiles rotate out via pool bufs)

            if l < L - 1:
                allgather(tabs[l + 1])

    nc.compile()
    return nc


# ============================ numpy emulation (debug) ============================

def _emulate(x, edge_index, W_in, b_in, Wc, bc, gamma, beta, rmean, rvar,
             W1, b1, W2, b2):
    """Mirror of the device dataflow in numpy (with bf16 rounding at the same
    points). Validates preprocessing + factorized algebra without hardware."""
    import scipy.sparse as sp
    meta, per_core, shared, b2f = _prep(
        x, edge_index, W_in, b_in, Wc, bc, gamma, beta, rmean, rvar,
        W1, b1, W2, b2)
    slot_of, node_of = meta["slot_of"], meta["node_of"]

    def q(a):
        return a.astype(BF16).astype(np.float32)

    ei = np.asarray(edge_index, np.int64)
    src = np.concatenate([ei[0], np.arange(N, dtype=np.int64)])
    dst = np.concatenate([ei[1], np.arange(N, dtype=np.int64)])
    gs, gd = slot_of[src], slot_of[dst]
    A = sp.csr_matrix((np.ones(gs.size, np.float32), (gd, gs)),
                      shape=(NTOT, NTOT))

    deg = np.bincount(dst, minlength=N)
    dis = (1.0 / np.sqrt(deg.astype(np.float64))).astype(np.float32)
    dis_slot = np.zeros(NTOT, np.float32)
    dis_slot[slot_of] = dis
    dis_b = q(dis_slot)

    xp = np.zeros((NTOT, DIN), np.float32)
    xp[slot_of] = np.asarray(x, np.float32)

    rs = 1.0 / np.sqrt(np.asarray(rvar, np.float64) + EPS)
    a_l = (rs * np.asarray(gamma, np.float64)).astype(np.float32)
    b_l = ((np.asarray(bc, np.float64) - np.asarray(rmean, np.float64))
           * rs * np.asarray(gamma, np.float64)
           + np.asarray(beta, np.float64)).astype(np.float32)
    wcp = np.asarray(Wc, np.float32) * a_l[:, None, :]

    h0 = np.maximum(q(xp) @ q(np.asarray(W_in, np.float32)) + b_in, 0)
    tab = q(q(dis_b[:, None] * q(h0)) @ q(wcp[0]))
    h = None
    for l in range(L):
        aggr = A @ tab                       # sum of bf16 msgs, f32 accum
        t = dis_b[:, None] * aggr + b_l[l]
        hn = np.maximum(t, 0)
        h = hn if l == 0 else hn + h
        if l < L - 1:
            hd = q(q(h) * dis_b[:, None])
            tab = q(hd @ q(wcp[l + 1]))
    m1 = np.maximum(q(q(h)) @ q(np.asarray(W1, np.float32))
                    + np.asarray(b1, np.float32), 0)
    o = (q(m1) @ q(np.asarray(W2, np.float32))).reshape(-1) + b2f
    out = np.empty(N, np.float32)
    out[node_of[slot_of]] = o[slot_of]
    return out


# ============================ entry point ============================

def kernel(x, edge_index, W_in, b_in, Wc, bc, gamma, beta, rmean, rvar,
           W1, b1, W2, b2):
    global LAST_EXEC_NS, TRACE_DIR
    from concourse.bass_utils import run_bass_kernel_spmd

    meta, per_core, shared, b2f = _prep(
        x, edge_index, W_in, b_in, Wc, bc, gamma, beta, rmean, rvar,
        W1, b1, W2, b2)

    ck = ("k1", tuple(meta["D_A"]), tuple(meta["D_B"]), b2f)
    if ck not in _CACHE:
        _CACHE.clear()
        _CACHE[ck] = _build(meta, b2f)
    nc = _CACHE[ck]

    in_maps = []
    for c in range(NCORES):
        m = dict(per_core[c])
        m.update(shared)
        in_maps.append(m)

    kwargs = {}
    if TRACE:
        _install_ntff_hook()
        kwargs = dict(trace=True)
    res = run_bass_kernel_spmd(nc, in_maps, list(range(NCORES)), **kwargs)
    LAST_EXEC_NS = res.exec_time_ns
    if TRACE and res.instructions_and_trace is not None:
        TRACE_DIR = res.instructions_and_trace[1]

    node_of, slot_of = meta["node_of"], meta["slot_of"]
    out = np.empty(N, np.float32)
    for c in range(NCORES):
        o = np.asarray(res.results[c]["out"]).reshape(-1)
        sl = np.arange(c * NPC, (c + 1) * NPC)
        real = node_of[sl] >= 0
        out[node_of[sl[real]]] = o[real.nonzero()[0]]
    return out


def _install_ntff_hook():
    import sys, types
    try:
        import antenv.axon_hooks  # noqa: F401
        return
    except ImportError:
        pass
    from trn_agent_boot.trn_boot import _ntff_profile_via_ctypes
    mod = types.ModuleType("antenv.axon_hooks")
    _h = [None]
    mod.set_axon_ntff_profile_hook = lambda h: _h.__setitem__(0, h)
    mod.get_axon_ntff_profile_hook = lambda: _h[0]
    sys.modules["antenv.axon_hooks"] = mod
    import antenv
    antenv.axon_hooks = mod
    mod.set_axon_ntff_profile_hook(
        _ntff_profile_via_ctypes("/opt/axon/libaxon_pjrt.so"))


# revision 18
# speedup vs baseline: 1.6196x; 1.1211x over previous
"""Trainium2 Bass kernel for AtomExposureGNN (3-layer GCN, N=50000, E=800000).

Distribution: nodes are partitioned across 8 NeuronCores (graph parallel).
Host-side (free, untimed): self-loops, degrees, symmetric-norm factorization,
degree-balanced node permutation, per-dst-block padded neighbor index lists.
Device: per layer, each core gathers message rows hw[src] (dma_gather from its
DRAM copy of the table), accumulates them per 128-node dst block on the
TensorEngine (PSUM, with dis[dst] folded into the stationary diag operand),
applies the folded batchnorm affine + relu + residual, computes the next
table shard hw = (dis*h) @ Wc', and AllGathers shards into the next table.

Key algebra: with dis = deg^-1/2,
  agg[d] = sum_e dis[s]dis[d] (h Wc)[s] = dis[d] * sum_e ((dis*h) Wc)[s]
  bn(agg+bc) = agg*a + b  with a,b per-channel  ->  a folds into Wc columns.
"""

import numpy as np
import ml_dtypes

# ---- problem constants (hardcoded per harness contract) ----
N, E, DIN, H, L = 50000, 800000, 64, 128, 3
NCORES = 8
NPC = 6272            # nodes per core (49 * 128), >= ceil(50000/8)=6250
NB = NPC // 128       # 49 dst blocks per core
NTOT = NPC * NCORES   # 50176 global slots
HALF = NTOT // 2      # region A = slots [0, HALF) (cores 0-3), B = rest
ZTOK = 6250           # zero-token: first pad slot of core 0 / core 4 (local id)
EPS = 1e-5
TILE_CAP = 112        # max message tiles (128 tokens each) per gather group
BF16 = ml_dtypes.bfloat16

LAST_EXEC_NS = None
TRACE = False
TRACE_DIR = None
DEBUG_NO_COLLECTIVE = False   # replace AllGather with local DMA (wrong numerics)
GATHER_MAX_IDX = 32768        # split dma_gather calls to at most this many idxs
GATHER_QUEUES = 4             # spread gathers over this many SWDGE queues
MSG_BUFS = 3                  # msg pool double/triple buffering

_CACHE = {}


# ============================ host preprocessing ============================

def _prep(x, edge_index, W_in, b_in, Wc, bc, gamma, beta, rmean, rvar,
          W1, b1, W2, b2):
    x = np.asarray(x, np.float32)
    ei = np.asarray(edge_index, np.int64)
    src = np.concatenate([ei[0], np.arange(N, dtype=np.int64)])
    dst = np.concatenate([ei[1], np.arange(N, dtype=np.int64)])

    deg = np.bincount(dst, minlength=N)
    dis = (1.0 / np.sqrt(deg.astype(np.float64))).astype(np.float32)

    # node -> core: stripe the degree-sorted order for balance
    order = np.argsort(deg, kind="stable")
    core_of = np.empty(N, np.int64)
    core_of[order] = np.arange(N) % NCORES
    region_a = core_of < (NCORES // 2)

    # per-node count of in-edges whose src lands in region A
    a_cnt = np.bincount(dst[region_a[src]], minlength=N)

    # per-core local slot order: (deg, a_cnt) lexsort keeps per-block padding tight
    slot_of = np.empty(N, np.int64)
    for c in range(NCORES):
        nodes_c = np.flatnonzero(core_of == c)
        o = np.lexsort((a_cnt[nodes_c], deg[nodes_c]))
        slot_of[nodes_c[o]] = c * NPC + np.arange(nodes_c.size)
    node_of = np.full(NTOT, -1, np.int64)
    node_of[slot_of] = np.arange(N)

    gd = slot_of[dst]
    gs = slot_of[src]
    reg = (gs >= HALF).astype(np.int64)
    tok = gs - reg * HALF
    core_e = gd // NPC
    blk = (gd % NPC) // 128
    par = gd % 128

    key = ((core_e * NB + blk) * 2 + reg) * 128 + par
    cnt = np.bincount(key, minlength=NCORES * NB * 2 * 128)
    cnt = cnt.reshape(NCORES, NB, 2, 128)
    D_A = cnt[:, :, 0, :].max(axis=(0, 2)).astype(np.int64)   # [NB]
    D_B = cnt[:, :, 1, :].max(axis=(0, 2)).astype(np.int64)

    # contiguous greedy grouping of blocks for batched gathers
    dsum = D_A + D_B
    groups, cur, acc = [], [], 0
    for b in range(NB):
        if cur and acc + dsum[b] > TILE_CAP:
            groups.append(cur)
            cur, acc = [], 0
        cur.append(b)
        acc += int(dsum[b])
    groups.append(cur)

    # idx sequence layout: per group, A spans of its blocks then B spans
    offA = np.zeros(NB, np.int64)
    offB = np.zeros(NB, np.int64)
    off = 0
    for gblocks in groups:
        for b in gblocks:
            offA[b] = off
            off += D_A[b] * 128
        for b in gblocks:
            offB[b] = off
            off += D_B[b] * 128
    ltot = int(off)
    assert ltot % 128 == 0

    # fill token slots: edge with within-(core,blk,reg,par) rank j goes to
    # sequence position off{A,B}[blk] + j*128 + par
    ordk = np.argsort(key, kind="stable")
    ks = key[ordk]
    newrun = np.r_[True, ks[1:] != ks[:-1]]
    runstart = np.flatnonzero(newrun)
    runid = np.cumsum(newrun) - 1
    cc = np.arange(ks.size) - runstart[runid]

    idx_all = np.full((NCORES, ltot), ZTOK, np.int16)
    pos = np.where(reg[ordk] == 0, offA[blk[ordk]], offB[blk[ordk]])
    pos = pos + cc * 128 + par[ordk]
    idx_all[core_e[ordk], pos] = tok[ordk].astype(np.int16)

    idx_sb = np.empty((NCORES, 128, ltot // 16), np.int16)
    for c in range(NCORES):
        w = idx_all[c].reshape(ltot // 16, 16).T
        idx_sb[c] = np.tile(w, (8, 1))

    # per-slot dis / x
    dis_slot = np.zeros(NTOT, np.float32)
    dis_slot[slot_of] = dis
    xp = np.zeros((NTOT, DIN), np.float32)
    xp[slot_of] = x

    dcol = np.empty((NCORES, 128, NB), np.float32)
    ddiag = np.zeros((NCORES, 128, NB, 128), np.float32)
    xT = np.empty((NCORES, DIN, NPC), np.float32)
    p_i = np.arange(128)
    for c in range(NCORES):
        d = dis_slot[c * NPC:(c + 1) * NPC].reshape(NB, 128)
        dcol[c] = d.T
        ddiag[c][p_i[:, None], np.arange(NB)[None, :], p_i[:, None]] = d.T
        xT[c] = xp[c * NPC:(c + 1) * NPC].T
    ddiag = ddiag.reshape(NCORES, 128, NB * 128).astype(BF16)

    # folded batchnorm affine
    rs = 1.0 / np.sqrt(np.asarray(rvar, np.float64) + EPS)
    a_l = (rs * np.asarray(gamma, np.float64)).astype(np.float32)       # [L,H]
    b_l = ((np.asarray(bc, np.float64) - np.asarray(rmean, np.float64))
           * rs * np.asarray(gamma, np.float64)
           + np.asarray(beta, np.float64)).astype(np.float32)           # [L,H]
    wcp = (np.asarray(Wc, np.float32) * a_l[:, None, :])                # [L,H,H]
    wc_sb = np.concatenate([wcp[i] for i in range(L)], axis=1).astype(BF16)
    brep = np.tile(b_l[None, :, None, :], (128, 1, 4, 1))
    brep = brep.reshape(128, L * 4 * H).astype(np.float32)

    assert (D_A + D_B >= 1).all()
    meta = dict(D_A=D_A, D_B=D_B, groups=groups, offA=offA, offB=offB,
                ltot=ltot, node_of=node_of, slot_of=slot_of)
    per_core = []
    for c in range(NCORES):
        per_core.append(dict(
            xT=np.ascontiguousarray(xT[c]),
            idx=np.ascontiguousarray(idx_sb[c]),
            ddiag=np.ascontiguousarray(ddiag[c]),
            dcol=np.ascontiguousarray(dcol[c]),
        ))
    shared = dict(
        win=np.asarray(W_in, np.float32).astype(BF16),                  # [64,128]
        wc=wc_sb,                                                       # [128,3H]
        w1=np.asarray(W1, np.float32).astype(BF16),                     # [128,64]
        w2=np.asarray(W2, np.float32).astype(BF16),                     # [64,1]
        bin=np.asarray(b_in, np.float32).reshape(H, 1),
        b1=np.asarray(b1, np.float32).reshape(H // 2, 1),
        brep=brep,
        ident=np.eye(128, dtype=np.float32).astype(BF16),
    )
    b2f = float(np.asarray(b2, np.float32).reshape(-1)[0])
    return meta, per_core, shared, b2f


# ============================ device kernel ============================

def _build(meta, b2f):
    import concourse.bass as bass
    import concourse.bacc as bacc
    import concourse.tile as tile
    import concourse.mybir as mybir
    from contextlib import ExitStack

    f32 = mybir.dt.float32
    bf16 = mybir.dt.bfloat16
    i16 = mybir.dt.int16
    RELU = mybir.ActivationFunctionType.Relu

    D_A, D_B = meta["D_A"], meta["D_B"]
    groups = meta["groups"]
    offA, offB = meta["offA"], meta["offB"]
    ltot = meta["ltot"]

    group_of = {}
    for g, gblocks in enumerate(groups):
        for b in gblocks:
            group_of[b] = g

    nc = bacc.Bacc("TRN2", target_bir_lowering=False, debug=False,
                   num_devices=NCORES, num_swdge_queues=max(1, GATHER_QUEUES))

    xT_d = nc.dram_tensor("xT", [DIN, NPC], f32, kind="ExternalInput")
    idx_d = nc.dram_tensor("idx", [128, ltot // 16], i16, kind="ExternalInput")
    ddiag_d = nc.dram_tensor("ddiag", [128, NB * 128], bf16, kind="ExternalInput")
    dcol_d = nc.dram_tensor("dcol", [128, NB], f32, kind="ExternalInput")
    win_d = nc.dram_tensor("win", [DIN, H], bf16, kind="ExternalInput")
    wc_d = nc.dram_tensor("wc", [H, L * H], bf16, kind="ExternalInput")
    w1_d = nc.dram_tensor("w1", [H, H // 2], bf16, kind="ExternalInput")
    w2_d = nc.dram_tensor("w2", [H // 2, 1], bf16, kind="ExternalInput")
    bin_d = nc.dram_tensor("bin", [H, 1], f32, kind="ExternalInput")
    b1_d = nc.dram_tensor("b1", [H // 2, 1], f32, kind="ExternalInput")
    brep_d = nc.dram_tensor("brep", [128, L * 4 * H], f32, kind="ExternalInput")
    ident_d = nc.dram_tensor("ident", [128, 128], bf16, kind="ExternalInput")
    out_d = nc.dram_tensor("out", [NB, 128], f32, kind="ExternalOutput")

    hwb = nc.dram_tensor("hwb", [NPC, H], bf16)
    tabs = [nc.dram_tensor(f"tab{i}", [NTOT, H], bf16, addr_space="Shared")
            for i in range(L)]
    rg = [list(range(NCORES))]

    with tile.TileContext(nc) as tc, ExitStack() as ctx:
        const = ctx.enter_context(tc.tile_pool(name="const", bufs=1))
        hpool = ctx.enter_context(tc.tile_pool(name="hst", bufs=1))
        msgp = ctx.enter_context(tc.tile_pool(name="msg", bufs=MSG_BUFS))
        work = ctx.enter_context(tc.tile_pool(name="work", bufs=3))
        psagg = ctx.enter_context(
            tc.tile_pool(name="psagg", bufs=2, space="PSUM"))
        psw = ctx.enter_context(tc.tile_pool(name="psw", bufs=2, space="PSUM"))

        # ---- persistent SBUF residents ----
        idx_t = const.tile([128, ltot // 16], i16)
        nc.sync.dma_start(idx_t[:], idx_d[:])
        ddiag_t = const.tile([128, NB * 128], bf16)
        nc.sync.dma_start(ddiag_t[:], ddiag_d[:])
        dcol_t = const.tile([128, NB], f32)
        nc.sync.dma_start(dcol_t[:], dcol_d[:])
        brep_t = const.tile([128, L * 4 * H], f32)
        nc.sync.dma_start(brep_t[:], brep_d[:])
        win_t = const.tile([DIN, H], bf16)
        nc.sync.dma_start(win_t[:], win_d[:])
        wc_t = const.tile([H, L * H], bf16)
        nc.sync.dma_start(wc_t[:], wc_d[:])
        w1_t = const.tile([H, H // 2], bf16)
        nc.sync.dma_start(w1_t[:], w1_d[:])
        w2_t = const.tile([H // 2, 1], bf16)
        nc.sync.dma_start(w2_t[:], w2_d[:])
        bin_t = const.tile([H, 1], f32)
        nc.sync.dma_start(bin_t[:], bin_d[:])
        b1_t = const.tile([H // 2, 1], f32)
        nc.sync.dma_start(b1_t[:], b1_d[:])
        ident_t = const.tile([128, 128], bf16)
        nc.sync.dma_start(ident_t[:], ident_d[:])

        h_store = hpool.tile([128, NB, H], f32)

        # ---- layer 0: hw0 = dis * (relu(x W_in + b_in) @ Wc'0), own shard ----
        done = 0
        l0chunks = [512] * (NPC // 512)
        if NPC % 512:
            l0chunks.append(NPC % 512)
        for w in l0chunks:
            sl = slice(done, done + w)
            xt_b = work.tile([DIN, w], bf16, tag="xt")
            nc.gpsimd.dma_start(xt_b[:], xT_d[:, sl])       # f32 -> bf16 cast
            ps0 = psw.tile([H, w], f32, tag="pT")
            nc.tensor.matmul(ps0[:], win_t[:], xt_b[:], start=True, stop=True)
            h0 = work.tile([H, w], bf16, tag="h0")
            nc.scalar.activation(h0[:], ps0[:], RELU, bias=bin_t[:])
            for s in range(w // 128):
                b = done // 128 + s
                ps2 = psw.tile([128, H], f32, tag="p2")
                nc.tensor.matmul(ps2[:], h0[:, s * 128:(s + 1) * 128],
                                 wc_t[:, 0:H], start=True, stop=True)
                hw_t = work.tile([128, H], bf16, tag="hwt")
                nc.vector.tensor_scalar_mul(hw_t[:], ps2[:],
                                            dcol_t[:, b:b + 1])
                nc.sync.dma_start(hwb[b * 128:(b + 1) * 128, :], hw_t[:])
            done += w

        def allgather(dst):
            if DEBUG_NO_COLLECTIVE:
                nc.gpsimd.dma_start(dst[0:NPC, :], hwb[:])
            else:
                nc.gpsimd.collective_compute(
                    "AllGather", mybir.AluOpType.bypass, replica_groups=rg,
                    ins=[hwb[:]], outs=[dst[:]])

        allgather(tabs[0])

        # ---- GCN layers ----
        supers = [list(range(s, min(s + 4, NB))) for s in range(0, NB, 4)]
        qn = [0]

        for l in range(L):
            tabA = tabs[l][:HALF, :]
            tabB = tabs[l][HALF:, :]

            msgs = {}

            def issue_group(g):
                gblocks = groups[g]
                la = int(sum(D_A[b] for b in gblocks))
                lb = int(sum(D_B[b] for b in gblocks))
                mA = mB = None
                step = max(1, GATHER_MAX_IDX // 128)
                if la:
                    mA = msgp.tile([128, la, H], bf16, tag="mA")
                    c0 = int(offA[gblocks[0]]) // 16
                    for t0 in range(0, la, step):
                        tn = min(step, la - t0)
                        nc.gpsimd.dma_gather(
                            mA[:, t0:t0 + tn, :], tabA,
                            idx_t[:, c0 + t0 * 8:c0 + (t0 + tn) * 8],
                            num_idxs=tn * 128, num_idxs_reg=tn * 128,
                            elem_size=H, elem_step=H, single_packet=False,
                            queue_num=qn[0] % max(1, GATHER_QUEUES))
                        qn[0] += 1
                if lb:
                    mB = msgp.tile([128, lb, H], bf16, tag="mB")
                    c0 = int(offB[gblocks[0]]) // 16
                    for t0 in range(0, lb, step):
                        tn = min(step, lb - t0)
                        nc.gpsimd.dma_gather(
                            mB[:, t0:t0 + tn, :], tabB,
                            idx_t[:, c0 + t0 * 8:c0 + (t0 + tn) * 8],
                            num_idxs=tn * 128, num_idxs_reg=tn * 128,
                            elem_size=H, elem_step=H, single_packet=False,
                            queue_num=qn[0] % max(1, GATHER_QUEUES))
                        qn[0] += 1
                msgs[g] = (mA, mB, gblocks[0])

            issue_group(0)
            for si, sblocks in enumerate(supers):
                w = len(sblocks)
                ps = psagg.tile([128, w, H], f32, tag="agg")
                for bi, b in enumerate(sblocks):
                    g = group_of[b]
                    if g not in msgs:
                        issue_group(g)
                    if g + 1 < len(groups) and (g + 1) not in msgs \
                            and b == groups[g][-1]:
                        issue_group(g + 1)
                    mA, mB, b0 = msgs[g]
                    jA = int(offA[b] - offA[b0]) // 128
                    jB0 = int(offB[b] - offA[b0]) // 128
                    la_g = int(sum(D_A[bb] for bb in groups[g]))
                    jB = int(offB[b] - offB[groups[g][0]]) // 128
                    chain = [(mA, jA + j) for j in range(int(D_A[b]))]
                    chain += [(mB, jB + j) for j in range(int(D_B[b]))]
                    nchain = len(chain)
                    for ci, (mt, jj) in enumerate(chain):
                        nc.tensor.matmul(
                            ps[:, bi, :],
                            ddiag_t[:, b * 128:(b + 1) * 128],
                            mt[:, jj, :],
                            start=(ci == 0), stop=(ci == nchain - 1))

                bsl = slice(sblocks[0], sblocks[0] + w)
                t = work.tile([128, w, H], f32, tag="ep")
                nc.vector.tensor_add(t[:], ps[:], brep_t[:].rearrange(
                    "p (l s c) -> p l s c", l=L, s=4)[:, l, :w, :])
                if l == 0:
                    nc.scalar.activation(h_store[:, bsl, :], t[:], RELU)
                else:
                    tmp = work.tile([128, w, H], f32, tag="rel")
                    nc.scalar.activation(tmp[:], t[:], RELU)
                    nc.vector.tensor_add(h_store[:, bsl, :], tmp[:],
                                         h_store[:, bsl, :])
                hb = work.tile([128, w, H], bf16, tag="hb")
                nc.vector.tensor_copy(hb[:], h_store[:, bsl, :])

                for s in range(w):
                    b = sblocks[0] + s
                    if l < L - 1:
                        pst = psw.tile([128, H], f32, tag="pT")
                        nc.tensor.matmul(pst[:], hb[:, s, :],
                                         ddiag_t[:, b * 128:(b + 1) * 128],
                                         start=True, stop=True)
                        hT = work.tile([128, H], bf16, tag="hT")
                        nc.vector.tensor_copy(hT[:], pst[:])
                        ps2 = psw.tile([128, H], f32, tag="p2")
                        nc.tensor.matmul(ps2[:], hT[:],
                                         wc_t[:, (l + 1) * H:(l + 2) * H],
                                         start=True, stop=True)
                        hw_t = work.tile([128, H], bf16, tag="hwt")
                        nc.vector.tensor_copy(hw_t[:], ps2[:])
                        nc.sync.dma_start(hwb[b * 128:(b + 1) * 128, :],
                                          hw_t[:])
                    else:
                        # final MLP on this block
                        pst = psw.tile([128, H], f32, tag="pT")
                        nc.tensor.matmul(pst[:], hb[:, s, :], ident_t[:],
                                         start=True, stop=True)
                        hT = work.tile([128, H], bf16, tag="hT")
                        nc.vector.tensor_copy(hT[:], pst[:])
                        psm = psw.tile([H // 2, H], f32, tag="p2")
                        nc.tensor.matmul(psm[:], w1_t[:], hT[:],
                                         start=True, stop=True)
                        m1 = work.tile([H // 2, H], bf16, tag="m1")
                        nc.scalar.activation(m1[:], psm[:], RELU,
                                             bias=b1_t[:])
                        pso = psw.tile([1, H], f32, tag="pT")
                        nc.tensor.matmul(pso[:], w2_t[:], m1[:],
                                         start=True, stop=True)
                        oseg = work.tile([1, H], f32, tag="oseg")
                        nc.vector.tensor_scalar_add(oseg[:], pso[:], b2f)
                        nc.sync.dma_start(out_d[b:b + 1, :], oseg[:])
                # release consumed groups (tiles rotate out via pool bufs)

            if l < L - 1:
                allgather(tabs[l + 1])

    nc.compile()
    return nc


# ============================ numpy emulation (debug) ============================

def _emulate(x, edge_index, W_in, b_in, Wc, bc, gamma, beta, rmean, rvar,
             W1, b1, W2, b2):
    """Mirror of the device dataflow in numpy (with bf16 rounding at the same
    points). Validates preprocessing + factorized algebra without hardware."""
    import scipy.sparse as sp
    meta, per_core, shared, b2f = _prep(
        x, edge_index, W_in, b_in, Wc, bc, gamma, beta, rmean, rvar,
        W1, b1, W2, b2)
    slot_of, node_of = meta["slot_of"], meta["node_of"]

    def q(a):
        return a.astype(BF16).astype(np.float32)

    ei = np.asarray(edge_index, np.int64)
    src = np.concatenate([ei[0], np.arange(N, dtype=np.int64)])
    dst = np.concatenate([ei[1], np.arange(N, dtype=np.int64)])
    gs, gd = slot_of[src], slot_of[dst]
    A = sp.csr_matrix((np.ones(gs.size, np.float32), (gd, gs)),
                      shape=(NTOT, NTOT))

    deg = np.bincount(dst, minlength=N)
    dis = (1.0 / np.sqrt(deg.astype(np.float64))).astype(np.float32)
    dis_slot = np.zeros(NTOT, np.float32)
    dis_slot[slot_of] = dis
    dis_b = q(dis_slot)

    xp = np.zeros((NTOT, DIN), np.float32)
    xp[slot_of] = np.asarray(x, np.float32)

    rs = 1.0 / np.sqrt(np.asarray(rvar, np.float64) + EPS)
    a_l = (rs * np.asarray(gamma, np.float64)).astype(np.float32)
    b_l = ((np.asarray(bc, np.float64) - np.asarray(rmean, np.float64))
           * rs * np.asarray(gamma, np.float64)
           + np.asarray(beta, np.float64)).astype(np.float32)
    wcp = np.asarray(Wc, np.float32) * a_l[:, None, :]

    h0 = np.maximum(q(xp) @ q(np.asarray(W_in, np.float32)) + b_in, 0)
    tab = q(q(dis_b[:, None] * q(h0)) @ q(wcp[0]))
    h = None
    for l in range(L):
        aggr = A @ tab                       # sum of bf16 msgs, f32 accum
        t = dis_b[:, None] * aggr + b_l[l]
        hn = np.maximum(t, 0)
        h = hn if l == 0 else hn + h
        if l < L - 1:
            hd = q(q(h) * dis_b[:, None])
            tab = q(hd @ q(wcp[l + 1]))
    m1 = np.maximum(q(q(h)) @ q(np.asarray(W1, np.float32))
                    + np.asarray(b1, np.float32), 0)
    o = (q(m1) @ q(np.asarray(W2, np.float32))).reshape(-1) + b2f
    out = np.empty(N, np.float32)
    out[node_of[slot_of]] = o[slot_of]
    return out


# ============================ entry point ============================

def kernel(x, edge_index, W_in, b_in, Wc, bc, gamma, beta, rmean, rvar,
           W1, b1, W2, b2):
    global LAST_EXEC_NS, TRACE_DIR
    from concourse.bass_utils import run_bass_kernel_spmd

    meta, per_core, shared, b2f = _prep(
        x, edge_index, W_in, b_in, Wc, bc, gamma, beta, rmean, rvar,
        W1, b1, W2, b2)

    ck = ("k1", tuple(meta["D_A"]), tuple(meta["D_B"]), b2f)
    if ck not in _CACHE:
        _CACHE.clear()
        _CACHE[ck] = _build(meta, b2f)
    nc = _CACHE[ck]

    in_maps = []
    for c in range(NCORES):
        m = dict(per_core[c])
        m.update(shared)
        in_maps.append(m)

    kwargs = {}
    if TRACE:
        _install_ntff_hook()
        kwargs = dict(trace=True)
    res = run_bass_kernel_spmd(nc, in_maps, list(range(NCORES)), **kwargs)
    LAST_EXEC_NS = res.exec_time_ns
    if TRACE and res.instructions_and_trace is not None:
        TRACE_DIR = res.instructions_and_trace[1]

    node_of, slot_of = meta["node_of"], meta["slot_of"]
    out = np.empty(N, np.float32)
    for c in range(NCORES):
        o = np.asarray(res.results[c]["out"]).reshape(-1)
        sl = np.arange(c * NPC, (c + 1) * NPC)
        real = node_of[sl] >= 0
        out[node_of[sl[real]]] = o[real.nonzero()[0]]
    return out


def _install_ntff_hook():
    import sys, types
    try:
        import antenv.axon_hooks  # noqa: F401
        return
    except ImportError:
        pass
    from trn_agent_boot.trn_boot import _ntff_profile_via_ctypes
    mod = types.ModuleType("antenv.axon_hooks")
    _h = [None]
    mod.set_axon_ntff_profile_hook = lambda h: _h.__setitem__(0, h)
    mod.get_axon_ntff_profile_hook = lambda: _h[0]
    sys.modules["antenv.axon_hooks"] = mod
    import antenv
    antenv.axon_hooks = mod
    mod.set_axon_ntff_profile_hook(
        _ntff_profile_via_ctypes("/opt/axon/libaxon_pjrt.so"))
